# revision 25
# baseline (speedup 1.0000x reference)
"""Trainium2 Bass kernel for nn_GroundPropagation (optimized v3).

Phase 1 (device): host pre-swizzles 80 h-rows of x to (hw-on-partition,
channel-on-free) bf16; ACT computes sigmoid; PE runs 200 accumulating
matmuls where moving = [4 ramp cols | 128 s cols] against the s-block
stationary, yielding per-channel disparity dots, sum(s) and the Gram
diagonal ||s||^2 in one PSUM accumulator. The remaining 16 h-rows stay
channel-major: ACT sigmoid, DVE row-sums + square-accumulate. Host
combines per-core partials in f64 and ranks channels.

Phase 2 (device): layout (wq*32+ch partition, (w', h_rev) free). Host
sends r = m*(sel_below - sel) (bf16) and mask (u8); the bottom-up scan
v = m*v + r then directly yields d = prop - sel -- no subtract pass and
no sel input. DVE/Pool tree-reduce |d| for the per-channel clip max;
after a tiny PE-assisted barrier the reciprocal is folded into a
diagonal-matrix PE transpose (y = d^T * diag(rcp), bf16 PSUM), ACT
copies y to SBUF, DVE/Pool tree-max over channels gives w, ACT
broadcasts it, DVE blends w*y. Host multiplies by den_c = 0.3*max|d_c|,
adds sel and scatters.
"""

import sys

sys.path.insert(0, "/opt/trn_rl_repo")

import numpy as np
import ml_dtypes

B, C, H, W = 8, 128, 96, 320
HW = H * W                  # 30720
NSEL = 16
NS = 2 * NSEL               # 32 selected channels
CLIP = 0.3
EPS = 1e-6
N_CORES = 8

# phase 1 split: first HSPL h-rows transposed for PE, rest channel-major
HSPL = 80                   # h rows in the transposed (PE) half
HWT = HSPL * W              # 25600 elems in PE half
JBS = [36, 36, 36, 36, 36, 20]   # j-blocks per transposed chunk (200 total)
NCHT = len(JBS)
SLOT = 132                  # moving width: 4 ramp + 128 s
HRS = [8, 8]                # h rows per channel-major chunk (16 total)
NCHN = len(HRS)
# phase 2
WQ = 4                      # w-quarters; partition p = wq*32 + ch
WPQ = W // WQ               # 80 columns per quarter
S2 = WPQ * H                # 7680 free elems per partition
NCH2 = 4                    # stage-I chunks (scan granularity)
CH2 = S2 // NCH2            # 1920
NSUP = 4                    # stage-II super-chunks
SUP = S2 // NSUP            # 1920

_cache = {}


def _runner(nc, n_cores):
    """Build a cached jitted callable for this Bass program via PJRT."""
    import jax
    from concourse import mybir
    from concourse.bass2jax import (
        _bass_exec_p,
        install_neuronx_cc_hook,
        partition_id_tensor,
    )
    from jax.sharding import Mesh, PartitionSpec
    from jax.experimental.shard_map import shard_map

    install_neuronx_cc_hook()
    partition_name = nc.partition_id_tensor.name if nc.partition_id_tensor else None

    in_names, out_names, out_avals = [], [], []
    for alloc in nc.m.functions[0].allocations:
        if not isinstance(alloc, mybir.MemoryLocationSet):
            continue
        name = alloc.memorylocations[0].name
        if alloc.kind == "ExternalInput":
            if name != partition_name:
                in_names.append(name)
        elif alloc.kind == "ExternalOutput":
            out_names.append(name)
            out_avals.append(
                jax.core.ShapedArray(
                    tuple(alloc.tensor_shape), mybir.dt.np(alloc.dtype)
                )
            )
    n_params = len(in_names)
    n_outs = len(out_avals)
    all_names = in_names + out_names + ([partition_name] if partition_name else [])
    donate = tuple(range(n_params, n_params + n_outs))

    def _body(*args):
        operands = list(args)
        if partition_name is not None:
            operands.append(partition_id_tensor())
        outs = _bass_exec_p.bind(
            *operands,
            out_avals=tuple(out_avals),
            in_names=tuple(all_names),
            out_names=tuple(out_names),
            lowering_input_output_aliases=(),
            sim_require_finite=True,
            sim_require_nnan=True,
            nc=nc,
        )
        return tuple(outs)

    devices = jax.devices()[:n_cores]
    mesh = Mesh(np.asarray(devices), ("core",))
    in_specs = (PartitionSpec("core"),) * (n_params + n_outs)
    out_specs = (PartitionSpec("core"),) * n_outs
    sharded = jax.jit(
        shard_map(
            _body, mesh=mesh, in_specs=in_specs, out_specs=out_specs, check_rep=False
        ),
        donate_argnums=donate,
        keep_unused=True,
    )

    def run(in_maps):
        concat_in = [
            np.concatenate([np.asarray(m[name]) for m in in_maps], axis=0)
            for name in in_names
        ]
        zeros = [
            np.zeros((n_cores * a.shape[0], *a.shape[1:]), a.dtype) for a in out_avals
        ]
        out_arrs = sharded(*concat_in, *zeros)
        return [
            {
                name: np.asarray(out_arrs[i]).reshape(
                    n_cores, *out_avals[i].shape
                )[c]
                for i, name in enumerate(out_names)
            }
            for c in range(n_cores)
        ]

    return run


def build_phase1():
    from contextlib import ExitStack

    import concourse.tile as tile
    from concourse import bacc, mybir

    f32 = mybir.dt.float32
    bf16 = mybir.dt.bfloat16
    Alu = mybir.AluOpType
    Act = mybir.ActivationFunctionType
    nc = bacc.Bacc("TRN2", target_bir_lowering=False, debug=False,
                   num_devices=N_CORES)
    xts = nc.dram_tensor("xts", (C, HWT), bf16, kind="ExternalInput").ap()
    xn = nc.dram_tensor("xn", (C, HW - HWT), bf16, kind="ExternalInput").ap()
    rmp = nc.dram_tensor("rmp", (C, HWT // 128 * 4), bf16,
                         kind="ExternalInput").ap()
    st1 = nc.dram_tensor("st1", (C, SLOT), f32, kind="ExternalOutput").ap()
    rows = nc.dram_tensor("rows", (C, H - HSPL), f32,
                          kind="ExternalOutput").ap()
    ssqn = nc.dram_tensor("ssqn", (C, NCHN), f32, kind="ExternalOutput").ap()

    with tile.TileContext(nc) as tc, ExitStack() as ctx:
        px = ctx.enter_context(tc.tile_pool(name="px", bufs=2))
        pst = ctx.enter_context(tc.tile_pool(name="pst", bufs=2))
        pxn = ctx.enter_context(tc.tile_pool(name="pxn", bufs=2))
        psn = ctx.enter_context(tc.tile_pool(name="psn", bufs=2))
        psq = ctx.enter_context(tc.tile_pool(name="psq", bufs=2))
        psm = ctx.enter_context(tc.tile_pool(name="psm", bufs=1))
        pps = ctx.enter_context(tc.tile_pool(name="pps", bufs=1, space="PSUM"))

        rt = psm.tile([C, HWT // 128 * 4], bf16)
        nc.sync.dma_start(rt[:], rmp[:])
        rtv = rt[:].rearrange("p (j r) -> p j r", r=4)
        osb = psm.tile([C, SLOT], f32)
        rowsb = psm.tile([C, H - HSPL], f32)
        ssqsb = psm.tile([C, NCHN], f32)
        acc = pps.tile([C, SLOT], f32)
        jofs = [sum(JBS[:k]) for k in range(NCHT + 1)]
        hofs = [sum(HRS[:k]) for k in range(NCHN + 1)]
        NTOT = sum(JBS)
        # interleave: j0 n0 j1 n1 j2 j3 j4 j5 so DVE work starts early and
        # the PE/DVE tails overlap the last ACT chunks
        order = ["j0", "n0", "j1", "n1", "j2", "j3", "j4", "j5"]
        for key in order:
            i = int(key[1:])
            if key[0] == "n":
                hr = HRS[i]
                cw = hr * W
                xcn = pxn.tile([C, cw], bf16, tag="xcn",
                               padded_shape=[C, max(HRS) * W])
                nc.sync.dma_start(xcn[:], xn[:, hofs[i] * W:hofs[i + 1] * W])
                sn_ = psn.tile([C, cw], bf16, tag="sn",
                               padded_shape=[C, max(HRS) * W])
                nc.scalar.activation(sn_[:], xcn[:], Act.Sigmoid)
                nc.vector.tensor_reduce(
                    rowsb[:, hofs[i]:hofs[i + 1]],
                    sn_[:].rearrange("p (h w) -> p h w", w=W),
                    mybir.AxisListType.X, Alu.add)
                sq = psq.tile([C, cw], bf16, tag="sq",
                              padded_shape=[C, max(HRS) * W])
                nc.vector.scalar_tensor_tensor(
                    sq[:], sn_[:], 1.0, sn_[:], op0=Alu.mult, op1=Alu.mult,
                    accum_out=ssqsb[:, i:i + 1])
            else:
                jb = JBS[i]
                xc = px.tile([C, jb * 128], bf16, tag="xc",
                             padded_shape=[C, max(JBS) * 128])
                nc.sync.dma_start(xc[:], xts[:, jofs[i] * 128:jofs[i + 1] * 128])
                st = pst.tile([C, jb * SLOT], bf16, tag="st",
                              padded_shape=[C, max(JBS) * SLOT])
                stv = st[:].rearrange("p (j s) -> p j s", s=SLOT)
                nc.vector.tensor_copy(stv[:, :, 0:4],
                                      rtv[:, jofs[i]:jofs[i + 1], :])
                nc.scalar.activation(
                    stv[:, :, 4:SLOT],
                    xc[:].rearrange("p (j c) -> p j c", c=128),
                    Act.Sigmoid,
                )
                for j in range(jb):
                    g = jofs[i] + j
                    nc.tensor.matmul(
                        acc[:],
                        st[:, j * SLOT + 4:(j + 1) * SLOT],
                        st[:, j * SLOT:(j + 1) * SLOT],
                        start=(g == 0), stop=(g == NTOT - 1),
                    )
        nc.vector.tensor_copy(osb[:], acc[:])
        nc.sync.dma_start(st1[:], osb[:])
        nc.sync.dma_start(rows[:], rowsb[:])
        nc.sync.dma_start(ssqn[:], ssqsb[:])
    nc.compile()
    return nc


def build_phase2():
    from contextlib import ExitStack

    import concourse.tile as tile
    from concourse import bacc, mybir
    from concourse.masks import make_identity

    f32 = mybir.dt.float32
    bf16 = mybir.dt.bfloat16
    u8 = mybir.dt.uint8
    Alu = mybir.AluOpType
    Act = mybir.ActivationFunctionType
    nc = bacc.Bacc("TRN2", target_bir_lowering=False, debug=False,
                   num_devices=N_CORES)
    rin = nc.dram_tensor("rin", (C, S2), bf16, kind="ExternalInput").ap()
    msk = nc.dram_tensor("msk", (C, S2), u8, kind="ExternalInput").ap()
    ref = nc.dram_tensor("ref", (C, S2), bf16, kind="ExternalOutput").ap()
    NT = SUP // 128             # transpose tiles per super-chunk

    with tile.TileContext(nc) as tc, ExitStack() as ctx:
        pools = {}
        for name, bufs in [("r", 2), ("u", 1), ("ad", 2), ("t1", 2),
                           ("t2", 2), ("ysb", 2), ("c1", 2), ("c2", 2),
                           ("wr", 2), ("wex", 2), ("tb", 2), ("sm", 1)]:
            pools[name] = ctx.enter_context(tc.tile_pool(name=name, bufs=bufs))
        ppsy = ctx.enter_context(tc.tile_pool(name="ppsy", bufs=1, space="PSUM"))
        ppst = ctx.enter_context(tc.tile_pool(name="ppst", bufs=2, space="PSUM"))
        psm = pools["sm"]

        identf = psm.tile([C, C], f32)
        make_identity(nc, identf[:])
        identb = psm.tile([C, C], bf16)
        make_identity(nc, identb[:])
        mxp = psm.tile([C, NCH2], f32)
        mxr = psm.tile([C, 1], f32)
        Mc = psm.tile([1, NS], f32)
        den = psm.tile([1, NS], f32)
        rc = psm.tile([1, NS], f32)
        rc4 = psm.tile([1, C], bf16)
        one1 = psm.tile([1, 1], bf16)
        nc.vector.memset(one1[:], 1.0)
        rcp = psm.tile([C, 1], f32)
        diag = psm.tile([C, C], bf16)
        mt = psm.tile([C, S2], u8)
        nc.sync.dma_start(mt[:], msk[:])
        ut = pools["u"].tile([C, S2], bf16)

        # ---- stage I: per chunk scan -> d, and |d| absmax tree ----
        for k in range(NCH2):
            ksl = slice(k * CH2, (k + 1) * CH2)
            rt_ = pools["r"].tile([C, CH2], bf16, tag="r")
            nc.sync.dma_start(rt_[:], rin[:, ksl])
            # d = scan: state = m*state + r  (bottom-up, per column)
            nc.vector.tensor_tensor_scan(
                ut[:, ksl], mt[:, ksl], rt_[:], 0.0,
                op0=Alu.mult, op1=Alu.add)
            ad = pools["ad"].tile([C, CH2], bf16, tag="ad")
            nc.scalar.activation(ad[:], ut[:, ksl], Act.Abs)
            t1 = pools["t1"].tile([C, CH2 // 2], bf16, tag="t1")
            nc.vector.tensor_tensor(
                t1[:], ad[:, :CH2 // 2], ad[:, CH2 // 2:], Alu.max)
            t2 = pools["t2"].tile([C, CH2 // 4], bf16, tag="t2")
            nc.vector.tensor_tensor(
                t2[:], t1[:, :CH2 // 4], t1[:, CH2 // 4:], Alu.max)
            nc.vector.tensor_reduce(
                mxp[:, k:k + 1], t2[:], mybir.AxisListType.X, Alu.max)

        # ---- barrier (PE-assisted, no DMA) ----
        nc.vector.tensor_reduce(mxr[:], mxp[:], mybir.AxisListType.X, Alu.max)
        # barrier PE outputs live in slices of the (bufs=1) y psum ring
        ybar = ppsy.tile([C, SUP], f32, tag="y")
        mrow_ps = ybar[0:1, 0:C]
        nc.tensor.transpose(mrow_ps, mxr[:], identf[:])
        nc.vector.tensor_reduce(
            Mc[:], mrow_ps.rearrange("o (q c) -> o c q", q=WQ),
            mybir.AxisListType.X, Alu.max)
        # m==0 -> |d|==0 everywhere for that channel, so any finite rcp works
        nc.vector.tensor_scalar(den[:], Mc[:], CLIP, 1e-20,
                                op0=Alu.mult, op1=Alu.max)
        nc.vector.reciprocal(rc[:], den[:])
        nc.vector.tensor_copy(
            rc4[:].rearrange("o (q c) -> o q c", q=WQ),
            rc[:].unsqueeze(1).broadcast_to((1, WQ, NS)))
        rcp_ps = ybar[:, 128:129]
        nc.tensor.matmul(rcp_ps, rc4[:], one1[:], start=True, stop=True)
        nc.vector.tensor_copy(rcp[:], rcp_ps)
        # diag(rcp) for the scaled PE transpose
        nc.vector.tensor_scalar(diag[:], identf[:], rcp[:], None, op0=Alu.mult)

        # ---- stage II per super-chunk: scaled transpose, chanmax, blend ----
        for g in range(NSUP):
            gsl = slice(g * SUP, (g + 1) * SUP)
            yps = ppsy.tile([C, SUP], f32, tag="y")
            utp = ppst.tile([C, SUP], bf16, tag="uT")
            for t in range(NT):
                o = g * SUP + t * 128
                # y = u^T @ diag(rcp): transpose + per-channel scale in one
                nc.tensor.matmul(yps[:, t * 128:(t + 1) * 128],
                                 ut[:, o:o + 128], diag[:],
                                 start=True, stop=True)
                # raw transposed d for the blend
                nc.tensor.transpose(utp[:, t * 128:(t + 1) * 128],
                                    ut[:, o:o + 128], identb[:])
            ysb = pools["ysb"].tile([C, SUP], bf16, tag="ysb")
            nc.scalar.activation(ysb[:], yps[:], Act.Abs)
            yv = ysb[:].rearrange("p (g c) -> p g c", c=NS)
            NG = SUP // NS              # 60 pixel-groups
            c1 = pools["c1"].tile([C, NG * 16], bf16, tag="c1")
            nc.vector.tensor_tensor(
                c1[:].rearrange("p (g c) -> p g c", c=16),
                yv[:, :, 0:16], yv[:, :, 16:32], Alu.max)
            c1v = c1[:].rearrange("p (g c) -> p g c", c=16)
            c2 = pools["c2"].tile([C, NG * 8], bf16, tag="c2")
            nc.vector.tensor_tensor(
                c2[:].rearrange("p (g c) -> p g c", c=8),
                c1v[:, :, 0:8], c1v[:, :, 8:16], Alu.max)
            wr = pools["wr"].tile([C, NG], bf16, tag="wr")
            nc.vector.tensor_reduce(
                wr[:], c2[:].rearrange("p (g c) -> p g c", c=8),
                mybir.AxisListType.X, Alu.max)
            nc.vector.tensor_scalar(wr[:], wr[:], 1.0, None, op0=Alu.min)
            # broadcast w over the 32 channel slots and blend with raw d^T
            wex = pools["wex"].tile([C, SUP], bf16, tag="wex")
            nc.scalar.activation(
                wex[:].rearrange("p (g c) -> p g c", c=NS),
                wr[:].unsqueeze(-1).broadcast_to((C, NG, NS)),
                Act.Copy)
            tb = pools["tb"].tile([C, SUP], bf16, tag="tb")
            nc.vector.tensor_tensor(tb[:], wex[:], utp[:], Alu.mult)
            nc.sync.dma_start(ref[:, gsl], tb[:])
    nc.compile()
    return nc


# ---------------- host side ----------------

def _dispfull_f32():
    disp = np.linspace(0.1, 1.0, H).astype(np.float32)
    return np.repeat(disp, W)                       # (HW,) hw = h*W + w


def _ramp_array():
    """(128, HWT//128 * 4) bf16: per j-block cols [disp_hi, disp_lo, 1, 0]."""
    df = _dispfull_f32()[:HWT]
    hi = df.astype(ml_dtypes.bfloat16)
    lo = (df - hi.astype(np.float32)).astype(ml_dtypes.bfloat16)
    nj = HWT // 128
    r = np.zeros((128, nj, 4), dtype=ml_dtypes.bfloat16)
    r[:, :, 0] = hi.reshape(nj, 128).T
    r[:, :, 1] = lo.reshape(nj, 128).T
    r[:, :, 2] = np.asarray(1.0, ml_dtypes.bfloat16)
    return np.ascontiguousarray(r).reshape(128, nj * 4)


def _pack_phase1(x):
    """(B,C,H,W) f32 -> transposed half (B,128,HWT) + normal half, bf16."""
    xb = x.astype(ml_dtypes.bfloat16)
    xt_half = xb[:, :, :HSPL, :].reshape(B, C, HWT // 128, 128)
    xts = np.ascontiguousarray(xt_half.transpose(0, 3, 2, 1)).reshape(B, 128, HWT)
    xn = np.ascontiguousarray(xb[:, :, HSPL:, :]).reshape(B, C, HW - HWT)
    return xts, xn


def _select_channels(st1_stack, rows_stack, ssqn_stack):
    """Combine per-core phase-1 outputs in f64 and rank channels."""
    blk = st1_stack.astype(np.float64)                     # (B, C, SLOT)
    dot_disp = blk[:, :, 0].sum(axis=0) + blk[:, :, 1].sum(axis=0)
    sum_s = blk[:, :, 2].sum(axis=0)
    g = blk[:, :, 4:]                                      # (B, C, C)
    ssq = np.einsum('bcc->c', g, optimize=True)
    # channel-major-half contributions
    rows = rows_stack.astype(np.float64).sum(axis=0)       # (C, H-HSPL)
    disp = np.linspace(0.1, 1.0, H).astype(np.float32).astype(np.float64)
    dot_disp += rows @ disp[HSPL:]
    sum_s += rows.sum(axis=1)
    ssq += ssqn_stack.astype(np.float64).sum(axis=(0, 2))
    dot_depth = sum_s - dot_disp
    df = _dispfull_f32().astype(np.float64)
    vn_disp = np.sqrt(B * (df @ df))
    vn_depth = np.sqrt(B * ((1.0 - df) @ (1.0 - df)))
    sn = np.maximum(np.sqrt(ssq), EPS)
    cos_disp = dot_disp / (sn * vn_disp)
    cos_depth = dot_depth / (sn * vn_depth)
    disp_idx = np.argsort(-cos_disp, kind="stable")[:NSEL]
    depth_idx = np.argsort(-cos_depth, kind="stable")[:NSEL]
    return np.concatenate([disp_idx, depth_idx])


def _pack_phase2_inputs(input_features, dynamic_masks, idx):
    """Pack r = m*(sel_below - sel) and mask into per-core (128, 7680)
    bf16/u8 layout: partition p = wq*32 + ch, free t = w'*96 + (95 - h)."""
    sel = input_features[:, idx]                       # (B, 32, H, W) f32
    m_r = (dynamic_masks[:, ::-1, :] != 0)             # (B, Hrev, W) bool
    m_r = m_r.copy()
    m_r[:, 0, :] = False            # force reset at each column's bottom row

    sel_rev = sel[:, :, ::-1, :]                       # (B, 32, Hrev, W)
    dsel = np.zeros_like(sel_rev)
    dsel[:, :, 1:] = sel_rev[:, :, :-1] - sel_rev[:, :, 1:]
    r = np.where(m_r[:, None], dsel, 0.0)              # (B, 32, Hrev, W)

    def pack(t, dtype):                                # (B,32,Hrev,W) -> (B,128,S2)
        tt = t.transpose(0, 1, 3, 2)                   # (B, 32, W, Hrev)
        tp = np.ascontiguousarray(
            tt.reshape(B, NS, WQ, WPQ, H).transpose(0, 2, 1, 3, 4)
        ).reshape(B, C, S2)
        return tp.astype(dtype)

    r_b = pack(r, ml_dtypes.bfloat16)
    m_big = np.broadcast_to(m_r[:, None], (B, NS, H, W))
    m_b = pack(m_big, np.uint8)
    sel_p = pack(sel_rev, np.float32)                  # for host blend
    return r_b, m_b, sel_p


def _unpack_phase2(tb_stack, sel_p):
    """(B,128,7680) bf16 w*d in transposed layout -> (B,32,H,W) refined."""
    NT2 = S2 // 128
    t = tb_stack.astype(np.float32).reshape(B, 128, NT2, 128)
    wd = t.transpose(0, 3, 2, 1).reshape(B, 128, S2)   # back to packed layout
    r = wd + sel_p                                     # refined, packed
    r = r.reshape(B, WQ, NS, WPQ, H).transpose(0, 2, 1, 3, 4)
    r = r.reshape(B, NS, W, H).transpose(0, 1, 3, 2)   # (B, 32, Hrev, W)
    return r[:, :, ::-1, :]


def _get_runners():
    if "run1" not in _cache:
        nc1 = build_phase1()
        _cache["run1"] = _runner(nc1, N_CORES)
        nc2 = build_phase2()
        _cache["run2"] = _runner(nc2, N_CORES)
    return _cache["run1"], _cache["run2"]


def _max_masked_run(dynamic_masks):
    m = (dynamic_masks != 0)
    best = np.zeros((B, W), dtype=np.int32)
    cur = np.zeros((B, W), dtype=np.int32)
    for h in range(H - 1, -1, -1):
        cur = np.where(m[:, h, :], cur + 1, 0)
        best = np.maximum(best, cur)
    return int(best.max())


def kernel(input_features, dynamic_masks):
    input_features = np.asarray(input_features, dtype=np.float32)
    dynamic_masks = np.asarray(dynamic_masks)
    run1, run2 = _get_runners()

    # Phase 1: per-channel reductions on device (PE half + DVE half)
    xts, xn = _pack_phase1(input_features)
    if "ramp" not in _cache:
        _cache["ramp"] = _ramp_array()
    ramp = _cache["ramp"]
    in_maps1 = [{"xts": xts[b], "xn": xn[b], "rmp": ramp} for b in range(B)]
    outs1 = run1(in_maps1)
    st1_stack = np.stack([o["st1"] for o in outs1])
    rows_stack = np.stack([o["rows"] for o in outs1])
    ssqn_stack = np.stack([o["ssqn"] for o in outs1])
    idx = _select_channels(st1_stack, rows_stack, ssqn_stack)

    # the single-scan propagation is exact iff no masked run >= 33
    assert _max_masked_run(dynamic_masks) <= 32, (
        "masked run of >= 33 rows: single-scan shortcut invalid for this input"
    )

    # Phase 2: propagation + blend on device (device returns w*d*rcp)
    r_b, m_b, sel_p = _pack_phase2_inputs(input_features, dynamic_masks, idx)
    in_maps2 = [{"rin": r_b[b], "msk": m_b[b]} for b in range(B)]
    outs2 = run2(in_maps2)
    tb_stack = np.stack([o["ref"] for o in outs2])
    refined = _unpack_phase2(tb_stack, sel_p)

    out = input_features.copy()
    out[:, idx] = refined
    return out


# revision 34
# speedup vs baseline: 1.0623x; 1.0623x over previous
"""Trainium2 Bass kernel for nn_GroundPropagation (optimized v3).

Phase 1 (device): host pre-swizzles 80 h-rows of x to (hw-on-partition,
channel-on-free) bf16; ACT computes sigmoid; PE runs 200 accumulating
matmuls where moving = [4 ramp cols | 128 s cols] against the s-block
stationary, yielding per-channel disparity dots, sum(s) and the Gram
diagonal ||s||^2 in one PSUM accumulator. The remaining 16 h-rows stay
channel-major: ACT sigmoid, DVE row-sums + square-accumulate. Host
combines per-core partials in f64 and ranks channels.

Phase 2 (device): layout (wq*32+ch partition, (w', h_rev) free). Host
sends r = m*(sel_below - sel) (bf16) and mask (u8); the bottom-up scan
v = m*v + r then directly yields d = prop - sel -- no subtract pass and
no sel input. DVE/Pool tree-reduce |d| for the per-channel clip max;
after a tiny PE-assisted barrier the reciprocal is folded into a
diagonal-matrix PE transpose (y = d^T * diag(rcp), bf16 PSUM), ACT
copies y to SBUF, DVE/Pool tree-max over channels gives w, ACT
broadcasts it, DVE blends w*y. Host multiplies by den_c = 0.3*max|d_c|,
adds sel and scatters.
"""

import sys

sys.path.insert(0, "/opt/trn_rl_repo")

import numpy as np
import ml_dtypes

B, C, H, W = 8, 128, 96, 320
HW = H * W                  # 30720
NSEL = 16
NS = 2 * NSEL               # 32 selected channels
CLIP = 0.3
EPS = 1e-6
N_CORES = 8

# phase 1 split: first HSPL h-rows transposed for PE, rest channel-major
HSPL = 80                   # h rows in the transposed (PE) half
HWT = HSPL * W              # 25600 elems in PE half
JBS = [36, 36, 36, 36, 36, 20]   # j-blocks per transposed chunk (200 total)
NCHT = len(JBS)
SLOT = 132                  # moving width: 4 ramp + 128 s
HRS = [8, 8]                # h rows per channel-major chunk (16 total)
NCHN = len(HRS)
# phase 2
WQ = 4                      # w-quarters; partition p = wq*32 + ch
WPQ = W // WQ               # 80 columns per quarter
S2 = WPQ * H                # 7680 free elems per partition
NCH2 = 4                    # stage-I chunks (scan granularity)
CH2 = S2 // NCH2            # 1920
NSUP = 5                    # stage-II super-chunks
SUP = S2 // NSUP            # 1536

_cache = {}


def _runner(nc, n_cores):
    """Build a cached jitted callable for this Bass program via PJRT."""
    import jax
    from concourse import mybir
    from concourse.bass2jax import (
        _bass_exec_p,
        install_neuronx_cc_hook,
        partition_id_tensor,
    )
    from jax.sharding import Mesh, PartitionSpec
    from jax.experimental.shard_map import shard_map

    install_neuronx_cc_hook()
    partition_name = nc.partition_id_tensor.name if nc.partition_id_tensor else None

    in_names, out_names, out_avals = [], [], []
    for alloc in nc.m.functions[0].allocations:
        if not isinstance(alloc, mybir.MemoryLocationSet):
            continue
        name = alloc.memorylocations[0].name
        if alloc.kind == "ExternalInput":
            if name != partition_name:
                in_names.append(name)
        elif alloc.kind == "ExternalOutput":
            out_names.append(name)
            out_avals.append(
                jax.core.ShapedArray(
                    tuple(alloc.tensor_shape), mybir.dt.np(alloc.dtype)
                )
            )
    n_params = len(in_names)
    n_outs = len(out_avals)
    all_names = in_names + out_names + ([partition_name] if partition_name else [])
    donate = tuple(range(n_params, n_params + n_outs))

    def _body(*args):
        operands = list(args)
        if partition_name is not None:
            operands.append(partition_id_tensor())
        outs = _bass_exec_p.bind(
            *operands,
            out_avals=tuple(out_avals),
            in_names=tuple(all_names),
            out_names=tuple(out_names),
            lowering_input_output_aliases=(),
            sim_require_finite=True,
            sim_require_nnan=True,
            nc=nc,
        )
        return tuple(outs)

    devices = jax.devices()[:n_cores]
    mesh = Mesh(np.asarray(devices), ("core",))
    in_specs = (PartitionSpec("core"),) * (n_params + n_outs)
    out_specs = (PartitionSpec("core"),) * n_outs
    sharded = jax.jit(
        shard_map(
            _body, mesh=mesh, in_specs=in_specs, out_specs=out_specs, check_rep=False
        ),
        donate_argnums=donate,
        keep_unused=True,
    )

    def run(in_maps):
        concat_in = [
            np.concatenate([np.asarray(m[name]) for m in in_maps], axis=0)
            for name in in_names
        ]
        zeros = [
            np.zeros((n_cores * a.shape[0], *a.shape[1:]), a.dtype) for a in out_avals
        ]
        out_arrs = sharded(*concat_in, *zeros)
        return [
            {
                name: np.asarray(out_arrs[i]).reshape(
                    n_cores, *out_avals[i].shape
                )[c]
                for i, name in enumerate(out_names)
            }
            for c in range(n_cores)
        ]

    return run


def build_phase1():
    from contextlib import ExitStack

    import concourse.tile as tile
    from concourse import bacc, mybir

    f32 = mybir.dt.float32
    bf16 = mybir.dt.bfloat16
    Alu = mybir.AluOpType
    Act = mybir.ActivationFunctionType
    nc = bacc.Bacc("TRN2", target_bir_lowering=False, debug=False,
                   num_devices=N_CORES)
    xts = nc.dram_tensor("xts", (C, HWT), bf16, kind="ExternalInput").ap()
    xn = nc.dram_tensor("xn", (C, HW - HWT), bf16, kind="ExternalInput").ap()
    rmp = nc.dram_tensor("rmp", (C, HWT // 128 * 4), bf16,
                         kind="ExternalInput").ap()
    st1 = nc.dram_tensor("st1", (C, SLOT), f32, kind="ExternalOutput").ap()
    rows = nc.dram_tensor("rows", (C, H - HSPL), f32,
                          kind="ExternalOutput").ap()
    ssqn = nc.dram_tensor("ssqn", (C, NCHN), f32, kind="ExternalOutput").ap()

    with tile.TileContext(nc) as tc, ExitStack() as ctx:
        px = ctx.enter_context(tc.tile_pool(name="px", bufs=2))
        pst = ctx.enter_context(tc.tile_pool(name="pst", bufs=2))
        pxn = ctx.enter_context(tc.tile_pool(name="pxn", bufs=2))
        psn = ctx.enter_context(tc.tile_pool(name="psn", bufs=2))
        psq = ctx.enter_context(tc.tile_pool(name="psq", bufs=2))
        psm = ctx.enter_context(tc.tile_pool(name="psm", bufs=1))
        pps = ctx.enter_context(tc.tile_pool(name="pps", bufs=1, space="PSUM"))

        rt = psm.tile([C, HWT // 128 * 4], bf16)
        nc.sync.dma_start(rt[:], rmp[:])
        rtv = rt[:].rearrange("p (j r) -> p j r", r=4)
        osb = psm.tile([C, SLOT], f32)
        rowsb = psm.tile([C, H - HSPL], f32)
        ssqsb = psm.tile([C, NCHN], f32)
        acc = pps.tile([C, SLOT], f32)
        jofs = [sum(JBS[:k]) for k in range(NCHT + 1)]
        hofs = [sum(HRS[:k]) for k in range(NCHN + 1)]
        NTOT = sum(JBS)
        # interleave: DVE work (n chunks) starts early; remaining j chunks
        # taper so the PE tail drains alongside the last ACT ops
        order = ["j0", "n0", "j1", "n1"] + [f"j{i}" for i in range(2, NCHT)]
        for key in order:
            i = int(key[1:])
            if key[0] == "n":
                hr = HRS[i]
                cw = hr * W
                xcn = pxn.tile([C, cw], bf16, tag="xcn",
                               padded_shape=[C, max(HRS) * W])
                nc.sync.dma_start(xcn[:], xn[:, hofs[i] * W:hofs[i + 1] * W])
                sn_ = psn.tile([C, cw], bf16, tag="sn",
                               padded_shape=[C, max(HRS) * W])
                nc.scalar.activation(sn_[:], xcn[:], Act.Sigmoid)
                nc.vector.tensor_reduce(
                    rowsb[:, hofs[i]:hofs[i + 1]],
                    sn_[:].rearrange("p (h w) -> p h w", w=W),
                    mybir.AxisListType.X, Alu.add)
                sq = psq.tile([C, cw], bf16, tag="sq",
                              padded_shape=[C, max(HRS) * W])
                nc.vector.scalar_tensor_tensor(
                    sq[:], sn_[:], 1.0, sn_[:], op0=Alu.mult, op1=Alu.mult,
                    accum_out=ssqsb[:, i:i + 1])
            else:
                jb = JBS[i]
                xc = px.tile([C, jb * 128], bf16, tag="xc",
                             padded_shape=[C, max(JBS) * 128])
                nc.sync.dma_start(xc[:], xts[:, jofs[i] * 128:jofs[i + 1] * 128])
                st = pst.tile([C, jb * SLOT], bf16, tag="st",
                              padded_shape=[C, max(JBS) * SLOT])
                stv = st[:].rearrange("p (j s) -> p j s", s=SLOT)
                nc.vector.tensor_copy(stv[:, :, 0:4],
                                      rtv[:, jofs[i]:jofs[i + 1], :])
                # halve the sigmoid so PE starts on the first half early
                jh = jb // 2
                for (a, b) in ((0, jh), (jh, jb)):
                    nc.scalar.activation(
                        stv[:, a:b, 4:SLOT],
                        xc[:].rearrange("p (j c) -> p j c", c=128)[:, a:b],
                        Act.Sigmoid,
                    )
                    for j in range(a, b):
                        g = jofs[i] + j
                        nc.tensor.matmul(
                            acc[:],
                            st[:, j * SLOT + 4:(j + 1) * SLOT],
                            st[:, j * SLOT:(j + 1) * SLOT],
                            start=(g == 0), stop=(g == NTOT - 1),
                        )
        nc.vector.tensor_copy(osb[:], acc[:])
        nc.sync.dma_start(st1[:], osb[:])
        nc.sync.dma_start(rows[:], rowsb[:])
        nc.sync.dma_start(ssqn[:], ssqsb[:])
    nc.compile()
    return nc


def build_phase2():
    from contextlib import ExitStack

    import concourse.tile as tile
    from concourse import bacc, mybir
    from concourse.masks import make_identity

    f32 = mybir.dt.float32
    bf16 = mybir.dt.bfloat16
    u8 = mybir.dt.uint8
    Alu = mybir.AluOpType
    Act = mybir.ActivationFunctionType
    nc = bacc.Bacc("TRN2", target_bir_lowering=False, debug=False,
                   num_devices=N_CORES)
    rin = nc.dram_tensor("rin", (C, S2), bf16, kind="ExternalInput").ap()
    msk = nc.dram_tensor("msk", (C, S2), u8, kind="ExternalInput").ap()
    ref = nc.dram_tensor("ref", (C, S2), bf16, kind="ExternalOutput").ap()
    NT = SUP // 128             # transpose tiles per super-chunk

    with tile.TileContext(nc) as tc, ExitStack() as ctx:
        pools = {}
        for name, bufs in [("r", 4), ("u", 1), ("m", 4), ("ad", 2), ("t1", 2),
                           ("t2", 2), ("ysb", 2), ("c1", 2), ("c2", 2),
                           ("wr", 2), ("wex", 2), ("tb", 2), ("sm", 1)]:
            pools[name] = ctx.enter_context(tc.tile_pool(name=name, bufs=bufs))
        ppsy = ctx.enter_context(tc.tile_pool(name="ppsy", bufs=2, space="PSUM"))
        ppst = ctx.enter_context(tc.tile_pool(name="ppst", bufs=1, space="PSUM"))
        psm = pools["sm"]

        identf = psm.tile([C, C], f32)
        make_identity(nc, identf[:])
        identb = psm.tile([C, C], bf16)
        make_identity(nc, identb[:])
        mxp = psm.tile([C, NCH2], f32)
        mxr = psm.tile([C, 1], f32)
        Mc = psm.tile([1, NS], f32)
        den = psm.tile([1, NS], f32)
        rc = psm.tile([1, NS], f32)
        rc4 = psm.tile([1, C], bf16)
        one1 = psm.tile([1, 1], bf16)
        nc.vector.memset(one1[:], 1.0)
        rcp = psm.tile([C, 1], f32)
        diag = psm.tile([C, C], bf16)
        ut = pools["u"].tile([C, S2], bf16)

        # ---- stage I: per chunk scan -> d, and |d| absmax tree ----
        for k in range(NCH2):
            ksl = slice(k * CH2, (k + 1) * CH2)
            mt = pools["m"].tile([C, CH2], u8, tag="m")
            nc.sync.dma_start(mt[:], msk[:, ksl])
            rt_ = pools["r"].tile([C, CH2], bf16, tag="r")
            nc.sync.dma_start(rt_[:], rin[:, ksl])
            # d = scan: state = m*state + r  (bottom-up, per column)
            nc.vector.tensor_tensor_scan(
                ut[:, ksl], mt[:], rt_[:], 0.0,
                op0=Alu.mult, op1=Alu.add)
            ad = pools["ad"].tile([C, CH2], bf16, tag="ad")
            nc.scalar.activation(ad[:], ut[:, ksl], Act.Abs)
            t1 = pools["t1"].tile([C, CH2 // 2], bf16, tag="t1")
            nc.vector.tensor_tensor(
                t1[:], ad[:, :CH2 // 2], ad[:, CH2 // 2:], Alu.max)
            t2 = pools["t2"].tile([C, CH2 // 4], bf16, tag="t2")
            nc.vector.tensor_tensor(
                t2[:], t1[:, :CH2 // 4], t1[:, CH2 // 4:], Alu.max)
            nc.vector.tensor_reduce(
                mxp[:, k:k + 1], t2[:], mybir.AxisListType.X, Alu.max)

        # ---- barrier (PE-assisted, no DMA) ----
        nc.vector.tensor_reduce(mxr[:], mxp[:], mybir.AxisListType.X, Alu.max)
        # barrier PE outputs live in slices of the (bufs=1) y psum ring
        ybar = ppsy.tile([C, SUP], f32, tag="y")
        mrow_ps = ybar[0:1, 0:C]
        nc.tensor.transpose(mrow_ps, mxr[:], identf[:])
        nc.vector.tensor_reduce(
            Mc[:], mrow_ps.rearrange("o (q c) -> o c q", q=WQ),
            mybir.AxisListType.X, Alu.max)
        # m==0 -> |d|==0 everywhere for that channel, so any finite rcp works
        nc.vector.tensor_scalar(den[:], Mc[:], CLIP, 1e-20,
                                op0=Alu.mult, op1=Alu.max)
        nc.vector.reciprocal(rc[:], den[:])
        nc.vector.tensor_copy(
            rc4[:].rearrange("o (q c) -> o q c", q=WQ),
            rc[:].unsqueeze(1).broadcast_to((1, WQ, NS)))
        rcp_ps = ybar[:, 128:129]
        nc.tensor.matmul(rcp_ps, rc4[:], one1[:], start=True, stop=True)
        nc.vector.tensor_copy(rcp[:], rcp_ps)
        # diag(rcp) for the scaled PE transpose
        nc.vector.tensor_scalar(diag[:], identf[:], rcp[:], None, op0=Alu.mult)

        # ---- stage II per super-chunk: scaled transpose, chanmax, blend ----
        for g in range(NSUP):
            gsl = slice(g * SUP, (g + 1) * SUP)
            yps = ppsy.tile([C, SUP], f32, tag="y")
            utp = ppst.tile([C, SUP], bf16, tag="uT")
            for t in range(NT):
                o = g * SUP + t * 128
                # y = u^T @ diag(rcp): transpose + per-channel scale in one
                nc.tensor.matmul(yps[:, t * 128:(t + 1) * 128],
                                 ut[:, o:o + 128], diag[:],
                                 start=True, stop=True)
                # raw transposed d for the blend
                nc.tensor.transpose(utp[:, t * 128:(t + 1) * 128],
                                    ut[:, o:o + 128], identb[:])
            ysb = pools["ysb"].tile([C, SUP], bf16, tag="ysb")
            nc.scalar.activation(ysb[:], yps[:], Act.Abs)
            yv = ysb[:].rearrange("p (g c) -> p g c", c=NS)
            NG = SUP // NS              # 60 pixel-groups
            c1 = pools["c1"].tile([C, NG * 16], bf16, tag="c1")
            nc.vector.tensor_tensor(
                c1[:].rearrange("p (g c) -> p g c", c=16),
                yv[:, :, 0:16], yv[:, :, 16:32], Alu.max)
            c1v = c1[:].rearrange("p (g c) -> p g c", c=16)
            c2 = pools["c2"].tile([C, NG * 8], bf16, tag="c2")
            nc.vector.tensor_tensor(
                c2[:].rearrange("p (g c) -> p g c", c=8),
                c1v[:, :, 0:8], c1v[:, :, 8:16], Alu.max)
            wr = pools["wr"].tile([C, NG], bf16, tag="wr")
            nc.vector.tensor_reduce(
                wr[:], c2[:].rearrange("p (g c) -> p g c", c=8),
                mybir.AxisListType.X, Alu.max)
            nc.vector.tensor_scalar(wr[:], wr[:], 1.0, None, op0=Alu.min)
            # broadcast w over the 32 channel slots and blend with raw d^T
            wex = pools["wex"].tile([C, SUP], bf16, tag="wex")
            nc.scalar.activation(
                wex[:].rearrange("p (g c) -> p g c", c=NS),
                wr[:].unsqueeze(-1).broadcast_to((C, NG, NS)),
                Act.Copy)
            tb = pools["tb"].tile([C, SUP], bf16, tag="tb")
            nc.vector.tensor_tensor(tb[:], wex[:], utp[:], Alu.mult)
            nc.sync.dma_start(ref[:, gsl], tb[:])
    nc.compile()
    return nc


# ---------------- host side ----------------

def _dispfull_f32():
    disp = np.linspace(0.1, 1.0, H).astype(np.float32)
    return np.repeat(disp, W)                       # (HW,) hw = h*W + w


def _ramp_array():
    """(128, HWT//128 * 4) bf16: per j-block cols [disp_hi, disp_lo, 1, 0]."""
    df = _dispfull_f32()[:HWT]
    hi = df.astype(ml_dtypes.bfloat16)
    lo = (df - hi.astype(np.float32)).astype(ml_dtypes.bfloat16)
    nj = HWT // 128
    r = np.zeros((128, nj, 4), dtype=ml_dtypes.bfloat16)
    r[:, :, 0] = hi.reshape(nj, 128).T
    r[:, :, 1] = lo.reshape(nj, 128).T
    r[:, :, 2] = np.asarray(1.0, ml_dtypes.bfloat16)
    return np.ascontiguousarray(r).reshape(128, nj * 4)


def _pack_phase1(x):
    """(B,C,H,W) f32 -> transposed half (B,128,HWT) + normal half, bf16."""
    xb = x.astype(ml_dtypes.bfloat16)
    xt_half = xb[:, :, :HSPL, :].reshape(B, C, HWT // 128, 128)
    xts = np.ascontiguousarray(xt_half.transpose(0, 3, 2, 1)).reshape(B, 128, HWT)
    xn = np.ascontiguousarray(xb[:, :, HSPL:, :]).reshape(B, C, HW - HWT)
    return xts, xn


def _select_channels(st1_stack, rows_stack, ssqn_stack):
    """Combine per-core phase-1 outputs in f64 and rank channels."""
    blk = st1_stack.astype(np.float64)                     # (B, C, SLOT)
    dot_disp = blk[:, :, 0].sum(axis=0) + blk[:, :, 1].sum(axis=0)
    sum_s = blk[:, :, 2].sum(axis=0)
    g = blk[:, :, 4:]                                      # (B, C, C)
    ssq = np.einsum('bcc->c', g, optimize=True)
    # channel-major-half contributions
    rows = rows_stack.astype(np.float64).sum(axis=0)       # (C, H-HSPL)
    disp = np.linspace(0.1, 1.0, H).astype(np.float32).astype(np.float64)
    dot_disp += rows @ disp[HSPL:]
    sum_s += rows.sum(axis=1)
    ssq += ssqn_stack.astype(np.float64).sum(axis=(0, 2))
    dot_depth = sum_s - dot_disp
    df = _dispfull_f32().astype(np.float64)
    vn_disp = np.sqrt(B * (df @ df))
    vn_depth = np.sqrt(B * ((1.0 - df) @ (1.0 - df)))
    sn = np.maximum(np.sqrt(ssq), EPS)
    cos_disp = dot_disp / (sn * vn_disp)
    cos_depth = dot_depth / (sn * vn_depth)
    disp_idx = np.argsort(-cos_disp, kind="stable")[:NSEL]
    depth_idx = np.argsort(-cos_depth, kind="stable")[:NSEL]
    return np.concatenate([disp_idx, depth_idx])


def _pack_phase2_inputs(input_features, dynamic_masks, idx):
    """Pack r = m*(sel_below - sel) and mask into per-core (128, 7680)
    bf16/u8 layout: partition p = wq*32 + ch, free t = w'*96 + (95 - h)."""
    sel = input_features[:, idx]                       # (B, 32, H, W) f32
    m_r = (dynamic_masks[:, ::-1, :] != 0)             # (B, Hrev, W) bool
    m_r = m_r.copy()
    m_r[:, 0, :] = False            # force reset at each column's bottom row

    sel_rev = sel[:, :, ::-1, :]                       # (B, 32, Hrev, W)
    dsel = np.zeros_like(sel_rev)
    dsel[:, :, 1:] = sel_rev[:, :, :-1] - sel_rev[:, :, 1:]
    r = np.where(m_r[:, None], dsel, 0.0)              # (B, 32, Hrev, W)

    def pack(t, dtype):                                # (B,32,Hrev,W) -> (B,128,S2)
        tt = t.transpose(0, 1, 3, 2)                   # (B, 32, W, Hrev)
        tp = np.ascontiguousarray(
            tt.reshape(B, NS, WQ, WPQ, H).transpose(0, 2, 1, 3, 4)
        ).reshape(B, C, S2)
        return tp.astype(dtype)

    r_b = pack(r, ml_dtypes.bfloat16)
    m_big = np.broadcast_to(m_r[:, None], (B, NS, H, W))
    m_b = pack(m_big, np.uint8)
    sel_p = pack(sel_rev, np.float32)                  # for host blend
    return r_b, m_b, sel_p


def _unpack_phase2(tb_stack, sel_p):
    """(B,128,7680) bf16 w*d in transposed layout -> (B,32,H,W) refined."""
    NT2 = S2 // 128
    t = tb_stack.astype(np.float32).reshape(B, 128, NT2, 128)
    wd = t.transpose(0, 3, 2, 1).reshape(B, 128, S2)   # back to packed layout
    r = wd + sel_p                                     # refined, packed
    r = r.reshape(B, WQ, NS, WPQ, H).transpose(0, 2, 1, 3, 4)
    r = r.reshape(B, NS, W, H).transpose(0, 1, 3, 2)   # (B, 32, Hrev, W)
    return r[:, :, ::-1, :]


def _get_runners():
    if "run1" not in _cache:
        nc1 = build_phase1()
        _cache["run1"] = _runner(nc1, N_CORES)
        nc2 = build_phase2()
        _cache["run2"] = _runner(nc2, N_CORES)
    return _cache["run1"], _cache["run2"]


def _max_masked_run(dynamic_masks):
    m = (dynamic_masks != 0)
    best = np.zeros((B, W), dtype=np.int32)
    cur = np.zeros((B, W), dtype=np.int32)
    for h in range(H - 1, -1, -1):
        cur = np.where(m[:, h, :], cur + 1, 0)
        best = np.maximum(best, cur)
    return int(best.max())


def kernel(input_features, dynamic_masks):
    input_features = np.asarray(input_features, dtype=np.float32)
    dynamic_masks = np.asarray(dynamic_masks)
    run1, run2 = _get_runners()

    # Phase 1: per-channel reductions on device (PE half + DVE half)
    xts, xn = _pack_phase1(input_features)
    if "ramp" not in _cache:
        _cache["ramp"] = _ramp_array()
    ramp = _cache["ramp"]
    in_maps1 = [{"xts": xts[b], "xn": xn[b], "rmp": ramp} for b in range(B)]
    outs1 = run1(in_maps1)
    st1_stack = np.stack([o["st1"] for o in outs1])
    rows_stack = np.stack([o["rows"] for o in outs1])
    ssqn_stack = np.stack([o["ssqn"] for o in outs1])
    idx = _select_channels(st1_stack, rows_stack, ssqn_stack)

    # the single-scan propagation is exact iff no masked run >= 33
    assert _max_masked_run(dynamic_masks) <= 32, (
        "masked run of >= 33 rows: single-scan shortcut invalid for this input"
    )

    # Phase 2: propagation + blend on device (device returns w*d*rcp)
    r_b, m_b, sel_p = _pack_phase2_inputs(input_features, dynamic_masks, idx)
    in_maps2 = [{"rin": r_b[b], "msk": m_b[b]} for b in range(B)]
    outs2 = run2(in_maps2)
    tb_stack = np.stack([o["ref"] for o in outs2])
    refined = _unpack_phase2(tb_stack, sel_p)

    out = input_features.copy()
    out[:, idx] = refined
    return out


# revision 41
# speedup vs baseline: 1.0916x; 1.0276x over previous
"""Trainium2 Bass kernel for nn_GroundPropagation (optimized v3).

Phase 1 (device): host pre-swizzles 80 h-rows of x to (hw-on-partition,
channel-on-free) bf16; ACT computes sigmoid; PE runs 200 accumulating
matmuls where moving = [4 ramp cols | 128 s cols] against the s-block
stationary, yielding per-channel disparity dots, sum(s) and the Gram
diagonal ||s||^2 in one PSUM accumulator. The remaining 16 h-rows stay
channel-major: ACT sigmoid, DVE row-sums + square-accumulate. Host
combines per-core partials in f64 and ranks channels.

Phase 2 (device): layout (wq*32+ch partition, (w', h_rev) free). Host
sends r = m*(sel_below - sel) (bf16) and mask (u8); the bottom-up scan
v = m*v + r then directly yields d = prop - sel -- no subtract pass and
no sel input. DVE/Pool tree-reduce |d| for the per-channel clip max;
after a tiny PE-assisted barrier the reciprocal is folded into a
diagonal-matrix PE transpose (y = d^T * diag(rcp), bf16 PSUM), ACT
copies y to SBUF, DVE/Pool tree-max over channels gives w, ACT
broadcasts it, DVE blends w*y. Host multiplies by den_c = 0.3*max|d_c|,
adds sel and scatters.
"""

import sys

sys.path.insert(0, "/opt/trn_rl_repo")

import numpy as np
import ml_dtypes

B, C, H, W = 8, 128, 96, 320
HW = H * W                  # 30720
NSEL = 16
NS = 2 * NSEL               # 32 selected channels
CLIP = 0.3
EPS = 1e-6
N_CORES = 8

# phase 1 split: first HSPL h-rows transposed for PE, rest channel-major
HSPL = 72                   # h rows in the transposed (PE) half
HWT = HSPL * W              # 23040 elems in PE half
JBS = [36, 36, 36, 36, 24, 12]   # j-blocks per transposed chunk (180 total)
NCHT = len(JBS)
SLOT = 132                  # moving width: 4 ramp + 128 s
HRS = [6, 6, 6, 6]          # h rows per channel-major chunk (24 total)
ORDER = ["j0", "n0", "j1", "n1", "j2", "n2", "j3", "n3", "j4", "j5"]
NCHN = len(HRS)
# phase 2
WQ = 4                      # w-quarters; partition p = wq*32 + ch
WPQ = W // WQ               # 80 columns per quarter
S2 = WPQ * H                # 7680 free elems per partition
CHS = [960, 960, 1920, 1920, 1920]    # stage-I chunk lengths (scan grain)
NCH2 = len(CHS)
NSUP = 5                    # stage-II super-chunks
SUP = S2 // NSUP            # 1536

_cache = {}


def _runner(nc, n_cores):
    """Build a cached jitted callable for this Bass program via PJRT."""
    import jax
    from concourse import mybir
    from concourse.bass2jax import (
        _bass_exec_p,
        install_neuronx_cc_hook,
        partition_id_tensor,
    )
    from jax.sharding import Mesh, PartitionSpec
    from jax.experimental.shard_map import shard_map

    install_neuronx_cc_hook()
    partition_name = nc.partition_id_tensor.name if nc.partition_id_tensor else None

    in_names, out_names, out_avals = [], [], []
    for alloc in nc.m.functions[0].allocations:
        if not isinstance(alloc, mybir.MemoryLocationSet):
            continue
        name = alloc.memorylocations[0].name
        if alloc.kind == "ExternalInput":
            if name != partition_name:
                in_names.append(name)
        elif alloc.kind == "ExternalOutput":
            out_names.append(name)
            out_avals.append(
                jax.core.ShapedArray(
                    tuple(alloc.tensor_shape), mybir.dt.np(alloc.dtype)
                )
            )
    n_params = len(in_names)
    n_outs = len(out_avals)
    all_names = in_names + out_names + ([partition_name] if partition_name else [])
    donate = tuple(range(n_params, n_params + n_outs))

    def _body(*args):
        operands = list(args)
        if partition_name is not None:
            operands.append(partition_id_tensor())
        outs = _bass_exec_p.bind(
            *operands,
            out_avals=tuple(out_avals),
            in_names=tuple(all_names),
            out_names=tuple(out_names),
            lowering_input_output_aliases=(),
            sim_require_finite=True,
            sim_require_nnan=True,
            nc=nc,
        )
        return tuple(outs)

    devices = jax.devices()[:n_cores]
    mesh = Mesh(np.asarray(devices), ("core",))
    in_specs = (PartitionSpec("core"),) * (n_params + n_outs)
    out_specs = (PartitionSpec("core"),) * n_outs
    sharded = jax.jit(
        shard_map(
            _body, mesh=mesh, in_specs=in_specs, out_specs=out_specs, check_rep=False
        ),
        donate_argnums=donate,
        keep_unused=True,
    )

    def run(in_maps):
        concat_in = [
            np.concatenate([np.asarray(m[name]) for m in in_maps], axis=0)
            for name in in_names
        ]
        zeros = [
            np.zeros((n_cores * a.shape[0], *a.shape[1:]), a.dtype) for a in out_avals
        ]
        out_arrs = sharded(*concat_in, *zeros)
        return [
            {
                name: np.asarray(out_arrs[i]).reshape(
                    n_cores, *out_avals[i].shape
                )[c]
                for i, name in enumerate(out_names)
            }
            for c in range(n_cores)
        ]

    return run


def build_phase1():
    from contextlib import ExitStack

    import concourse.tile as tile
    from concourse import bacc, mybir

    f32 = mybir.dt.float32
    bf16 = mybir.dt.bfloat16
    Alu = mybir.AluOpType
    Act = mybir.ActivationFunctionType
    nc = bacc.Bacc("TRN2", target_bir_lowering=False, debug=False,
                   num_devices=N_CORES)
    xts = nc.dram_tensor("xts", (C, HWT), bf16, kind="ExternalInput").ap()
    xn = nc.dram_tensor("xn", (C, HW - HWT), bf16, kind="ExternalInput").ap()
    rmp = nc.dram_tensor("rmp", (C, HWT // 128 * 4), bf16,
                         kind="ExternalInput").ap()
    # single packed output: [gram SLOT | rowsums | per-chunk ssq]
    OUTW = SLOT + (H - HSPL) + NCHN
    st1 = nc.dram_tensor("st1", (C, OUTW), f32, kind="ExternalOutput").ap()

    with tile.TileContext(nc) as tc, ExitStack() as ctx:
        px = ctx.enter_context(tc.tile_pool(name="px", bufs=2))
        pst = ctx.enter_context(tc.tile_pool(name="pst", bufs=2))
        pxn = ctx.enter_context(tc.tile_pool(name="pxn", bufs=2))
        psn = ctx.enter_context(tc.tile_pool(name="psn", bufs=2))
        psq = ctx.enter_context(tc.tile_pool(name="psq", bufs=2))
        psm = ctx.enter_context(tc.tile_pool(name="psm", bufs=1))
        pps = ctx.enter_context(tc.tile_pool(name="pps", bufs=1, space="PSUM"))

        rt = psm.tile([C, HWT // 128 * 4], bf16)
        nc.sync.dma_start(rt[:], rmp[:])
        rtv = rt[:].rearrange("p (j r) -> p j r", r=4)
        outb = psm.tile([C, SLOT + (H - HSPL) + NCHN], f32)
        osb = outb[:, 0:SLOT]
        rowsb = outb[:, SLOT:SLOT + (H - HSPL)]
        ssqsb = outb[:, SLOT + (H - HSPL):]
        acc = pps.tile([C, SLOT], f32)
        jofs = [sum(JBS[:k]) for k in range(NCHT + 1)]
        hofs = [sum(HRS[:k]) for k in range(NCHN + 1)]
        NTOT = sum(JBS)
        # interleave: DVE work (n chunks) starts early; remaining j chunks
        # taper so the PE tail drains alongside the last ACT ops
        order = list(ORDER)
        for key in order:
            i = int(key[1:])
            if key[0] == "n":
                hr = HRS[i]
                cw = hr * W
                xcn = pxn.tile([C, cw], bf16, tag="xcn",
                               padded_shape=[C, max(HRS) * W])
                nc.sync.dma_start(xcn[:], xn[:, hofs[i] * W:hofs[i + 1] * W])
                sn_ = psn.tile([C, cw], bf16, tag="sn",
                               padded_shape=[C, max(HRS) * W])
                nc.scalar.activation(sn_[:], xcn[:], Act.Sigmoid)
                nc.vector.tensor_reduce(
                    rowsb[:, hofs[i]:hofs[i + 1]],
                    sn_[:].rearrange("p (h w) -> p h w", w=W),
                    mybir.AxisListType.X, Alu.add)
                sq = psq.tile([C, cw], bf16, tag="sq",
                              padded_shape=[C, max(HRS) * W])
                nc.vector.scalar_tensor_tensor(
                    sq[:], sn_[:], 1.0, sn_[:], op0=Alu.mult, op1=Alu.mult,
                    accum_out=ssqsb[:, i:i + 1])
            else:
                jb = JBS[i]
                xc = px.tile([C, jb * 128], bf16, tag="xc",
                             padded_shape=[C, max(JBS) * 128])
                nc.sync.dma_start(xc[:], xts[:, jofs[i] * 128:jofs[i + 1] * 128])
                st = pst.tile([C, jb * SLOT], bf16, tag="st",
                              padded_shape=[C, max(JBS) * SLOT])
                stv = st[:].rearrange("p (j s) -> p j s", s=SLOT)
                nc.vector.tensor_copy(stv[:, :, 0:4],
                                      rtv[:, jofs[i]:jofs[i + 1], :])
                nc.scalar.activation(
                    stv[:, :, 4:SLOT],
                    xc[:].rearrange("p (j c) -> p j c", c=128),
                    Act.Sigmoid,
                )
                for j in range(jb):
                    g = jofs[i] + j
                    nc.tensor.matmul(
                        acc[:],
                        st[:, j * SLOT + 4:(j + 1) * SLOT],
                        st[:, j * SLOT:(j + 1) * SLOT],
                        start=(g == 0), stop=(g == NTOT - 1),
                    )
        nc.vector.tensor_copy(osb, acc[:])
        nc.sync.dma_start(st1[:], outb[:])
    nc.compile()
    return nc


def build_phase2():
    from contextlib import ExitStack

    import concourse.tile as tile
    from concourse import bacc, mybir
    from concourse.masks import make_identity

    f32 = mybir.dt.float32
    bf16 = mybir.dt.bfloat16
    u8 = mybir.dt.uint8
    Alu = mybir.AluOpType
    Act = mybir.ActivationFunctionType
    nc = bacc.Bacc("TRN2", target_bir_lowering=False, debug=False,
                   num_devices=N_CORES)
    rin = nc.dram_tensor("rin", (C, S2), bf16, kind="ExternalInput").ap()
    msk = nc.dram_tensor("msk", (C, S2), u8, kind="ExternalInput").ap()
    ref = nc.dram_tensor("ref", (C, S2), bf16, kind="ExternalOutput").ap()
    NT = SUP // 128             # transpose tiles per super-chunk

    with tile.TileContext(nc) as tc, ExitStack() as ctx:
        pools = {}
        for name, bufs in [("r", 5), ("u", 1), ("m", 5), ("ad", 2), ("t1", 2),
                           ("t2", 2), ("ysb", 2), ("c1", 2), ("c2", 2),
                           ("wr", 2), ("wex", 2), ("tb", 2), ("sm", 1)]:
            pools[name] = ctx.enter_context(tc.tile_pool(name=name, bufs=bufs))
        ppsy = ctx.enter_context(tc.tile_pool(name="ppsy", bufs=2, space="PSUM"))
        ppst = ctx.enter_context(tc.tile_pool(name="ppst", bufs=1, space="PSUM"))
        psm = pools["sm"]

        identf = psm.tile([C, C], f32)
        make_identity(nc, identf[:])
        identb = psm.tile([C, C], bf16)
        make_identity(nc, identb[:])
        mxp = psm.tile([C, NCH2], f32)
        mxr = psm.tile([C, 1], f32)
        Mc = psm.tile([1, NS], f32)
        den = psm.tile([1, NS], f32)
        rc = psm.tile([1, NS], f32)
        rc4 = psm.tile([1, C], bf16)
        one1 = psm.tile([1, 1], bf16)
        nc.vector.memset(one1[:], 1.0)
        rcp = psm.tile([C, 1], f32)
        diag = psm.tile([C, C], bf16)
        ut = pools["u"].tile([C, S2], bf16)

        # ---- stage I: per chunk scan -> d, and |d| absmax tree ----
        # leading chunks are small so the first scan starts early; the raw
        # uT transposes for the first two supers run here (no rcp needed),
        # warming the PE and shrinking the post-barrier fill
        utps = {}
        done = 0
        next_ut = 0
        for k, ch in enumerate(CHS):
            ksl = slice(done, done + ch)
            mt = pools["m"].tile([C, ch], u8, tag="m",
                                 padded_shape=[C, max(CHS)])
            nc.sync.dma_start(mt[:], msk[:, ksl])
            rt_ = pools["r"].tile([C, ch], bf16, tag="r",
                                  padded_shape=[C, max(CHS)])
            nc.sync.dma_start(rt_[:], rin[:, ksl])
            # d = scan: state = m*state + r  (bottom-up, per column)
            nc.vector.tensor_tensor_scan(
                ut[:, ksl], mt[:], rt_[:], 0.0,
                op0=Alu.mult, op1=Alu.add)
            done += ch
            ad = pools["ad"].tile([C, ch], bf16, tag="ad",
                                  padded_shape=[C, max(CHS)])
            nc.scalar.activation(ad[:], ut[:, ksl], Act.Abs)
            t1 = pools["t1"].tile([C, ch // 2], bf16, tag="t1",
                                  padded_shape=[C, max(CHS) // 2])
            nc.vector.tensor_tensor(
                t1[:], ad[:, :ch // 2], ad[:, ch // 2:], Alu.max)
            t2 = pools["t2"].tile([C, ch // 4], bf16, tag="t2",
                                  padded_shape=[C, max(CHS) // 4])
            nc.vector.tensor_tensor(
                t2[:], t1[:, :ch // 4], t1[:, ch // 4:], Alu.max)
            nc.vector.tensor_reduce(
                mxp[:, k:k + 1], t2[:], mybir.AxisListType.X, Alu.max)


        # ---- barrier (PE-assisted, no DMA) ----
        nc.vector.tensor_reduce(mxr[:], mxp[:], mybir.AxisListType.X, Alu.max)
        # barrier PE outputs live in slices of the (bufs=1) y psum ring
        ybar = ppsy.tile([C, SUP], f32, tag="y")
        mrow_ps = ybar[0:1, 0:C]
        nc.tensor.transpose(mrow_ps, mxr[:], identf[:])
        nc.vector.tensor_reduce(
            Mc[:], mrow_ps.rearrange("o (q c) -> o c q", q=WQ),
            mybir.AxisListType.X, Alu.max)
        # m==0 -> |d|==0 everywhere for that channel, so any finite rcp works
        nc.vector.tensor_scalar(den[:], Mc[:], CLIP, 1e-20,
                                op0=Alu.mult, op1=Alu.max)
        nc.vector.reciprocal(rc[:], den[:])
        nc.vector.tensor_copy(
            rc4[:].rearrange("o (q c) -> o q c", q=WQ),
            rc[:].unsqueeze(1).broadcast_to((1, WQ, NS)))
        rcp_ps = ybar[:, 128:129]
        nc.tensor.matmul(rcp_ps, rc4[:], one1[:], start=True, stop=True)
        nc.vector.tensor_copy(rcp[:], rcp_ps)
        # diag(rcp) for the scaled PE transpose
        nc.vector.tensor_scalar(diag[:], identf[:], rcp[:], None, op0=Alu.mult)

        # ---- stage II per super-chunk: scaled transpose, chanmax, blend ----
        def emit_y(g):
            yps = ppsy.tile([C, SUP], f32, tag="y")
            for t in range(NT):
                o = g * SUP + t * 128
                # y = u^T @ diag(rcp): transpose + per-channel scale in one
                nc.tensor.matmul(yps[:, t * 128:(t + 1) * 128],
                                 ut[:, o:o + 128], diag[:],
                                 start=True, stop=True)
            ysb = pools["ysb"].tile([C, SUP], bf16, tag="ysb")
            nc.scalar.activation(ysb[:], yps[:], Act.Abs)
            return ysb

        NG = SUP // NS
        ysbs = {0: emit_y(0)}
        for g in range(NSUP):
            gsl = slice(g * SUP, (g + 1) * SUP)
            if g + 1 < NSUP:
                ysbs[g + 1] = emit_y(g + 1)
            ysb = ysbs.pop(g)
            yv = ysb[:].rearrange("p (g c) -> p g c", c=NS)
            c1 = pools["c1"].tile([C, NG * 16], bf16, tag="c1")
            nc.vector.tensor_tensor(
                c1[:].rearrange("p (g c) -> p g c", c=16),
                yv[:, :, 0:16], yv[:, :, 16:32], Alu.max)
            c1v = c1[:].rearrange("p (g c) -> p g c", c=16)
            c2 = pools["c2"].tile([C, NG * 8], bf16, tag="c2")
            nc.vector.tensor_tensor(
                c2[:].rearrange("p (g c) -> p g c", c=8),
                c1v[:, :, 0:8], c1v[:, :, 8:16], Alu.max)
            wr = pools["wr"].tile([C, NG], bf16, tag="wr")
            nc.vector.tensor_reduce(
                wr[:], c2[:].rearrange("p (g c) -> p g c", c=8),
                mybir.AxisListType.X, Alu.max)
            nc.vector.tensor_scalar(wr[:], wr[:], 1.0, None, op0=Alu.min)
            # raw d^T just-in-time for the blend
            utp = ppst.tile([C, SUP], bf16, tag="uT")
            for t in range(NT):
                o = g * SUP + t * 128
                nc.tensor.transpose(utp[:, t * 128:(t + 1) * 128],
                                    ut[:, o:o + 128], identb[:])
            # broadcast w over the 32 channel slots and blend with raw d^T;
            # the last super streams out in halves to shrink the tail
            wex = pools["wex"].tile([C, SUP], bf16, tag="wex")
            tb = pools["tb"].tile([C, SUP], bf16, tag="tb")
            parts = 2 if g == NSUP - 1 else 1
            hw_ = SUP // parts
            hg = NG // parts
            for h in range(parts):
                hsl = slice(h * hw_, (h + 1) * hw_)
                nc.scalar.activation(
                    wex[:, hsl].rearrange("p (g c) -> p g c", c=NS),
                    wr[:, h * hg:(h + 1) * hg].unsqueeze(-1)
                    .broadcast_to((C, hg, NS)),
                    Act.Copy)
                nc.vector.tensor_tensor(tb[:, hsl], wex[:, hsl], utp[:, hsl],
                                        Alu.mult)
                nc.sync.dma_start(
                    ref[:, g * SUP + h * hw_:g * SUP + (h + 1) * hw_],
                    tb[:, hsl])
    nc.compile()
    return nc


# ---------------- host side ----------------

def _dispfull_f32():
    disp = np.linspace(0.1, 1.0, H).astype(np.float32)
    return np.repeat(disp, W)                       # (HW,) hw = h*W + w


def _ramp_array():
    """(128, HWT//128 * 4) bf16: per j-block cols [disp_hi, disp_lo, 1, 0]."""
    df = _dispfull_f32()[:HWT]
    hi = df.astype(ml_dtypes.bfloat16)
    lo = (df - hi.astype(np.float32)).astype(ml_dtypes.bfloat16)
    nj = HWT // 128
    r = np.zeros((128, nj, 4), dtype=ml_dtypes.bfloat16)
    r[:, :, 0] = hi.reshape(nj, 128).T
    r[:, :, 1] = lo.reshape(nj, 128).T
    r[:, :, 2] = np.asarray(1.0, ml_dtypes.bfloat16)
    return np.ascontiguousarray(r).reshape(128, nj * 4)


def _pack_phase1(x):
    """(B,C,H,W) f32 -> transposed half (B,128,HWT) + normal half, bf16."""
    xb = x.astype(ml_dtypes.bfloat16)
    xt_half = xb[:, :, :HSPL, :].reshape(B, C, HWT // 128, 128)
    xts = np.ascontiguousarray(xt_half.transpose(0, 3, 2, 1)).reshape(B, 128, HWT)
    xn = np.ascontiguousarray(xb[:, :, HSPL:, :]).reshape(B, C, HW - HWT)
    return xts, xn


def _select_channels(st1_stack, rows_stack, ssqn_stack):
    """Combine per-core phase-1 outputs in f64 and rank channels."""
    blk = st1_stack.astype(np.float64)                     # (B, C, SLOT)
    dot_disp = blk[:, :, 0].sum(axis=0) + blk[:, :, 1].sum(axis=0)
    sum_s = blk[:, :, 2].sum(axis=0)
    g = blk[:, :, 4:]                                      # (B, C, C)
    ssq = np.einsum('bcc->c', g, optimize=True)
    # channel-major-half contributions
    rows = rows_stack.astype(np.float64).sum(axis=0)       # (C, H-HSPL)
    disp = np.linspace(0.1, 1.0, H).astype(np.float32).astype(np.float64)
    dot_disp += rows @ disp[HSPL:]
    sum_s += rows.sum(axis=1)
    ssq += ssqn_stack.astype(np.float64).sum(axis=(0, 2))
    dot_depth = sum_s - dot_disp
    df = _dispfull_f32().astype(np.float64)
    vn_disp = np.sqrt(B * (df @ df))
    vn_depth = np.sqrt(B * ((1.0 - df) @ (1.0 - df)))
    sn = np.maximum(np.sqrt(ssq), EPS)
    cos_disp = dot_disp / (sn * vn_disp)
    cos_depth = dot_depth / (sn * vn_depth)
    disp_idx = np.argsort(-cos_disp, kind="stable")[:NSEL]
    depth_idx = np.argsort(-cos_depth, kind="stable")[:NSEL]
    return np.concatenate([disp_idx, depth_idx])


def _pack_phase2_inputs(input_features, dynamic_masks, idx):
    """Pack r = m*(sel_below - sel) and mask into per-core (128, 7680)
    bf16/u8 layout: partition p = wq*32 + ch, free t = w'*96 + (95 - h)."""
    sel = input_features[:, idx]                       # (B, 32, H, W) f32
    m_r = (dynamic_masks[:, ::-1, :] != 0)             # (B, Hrev, W) bool
    m_r = m_r.copy()
    m_r[:, 0, :] = False            # force reset at each column's bottom row

    sel_rev = sel[:, :, ::-1, :]                       # (B, 32, Hrev, W)
    dsel = np.zeros_like(sel_rev)
    dsel[:, :, 1:] = sel_rev[:, :, :-1] - sel_rev[:, :, 1:]
    r = np.where(m_r[:, None], dsel, 0.0)              # (B, 32, Hrev, W)

    def pack(t, dtype):                                # (B,32,Hrev,W) -> (B,128,S2)
        tt = t.transpose(0, 1, 3, 2)                   # (B, 32, W, Hrev)
        tp = np.ascontiguousarray(
            tt.reshape(B, NS, WQ, WPQ, H).transpose(0, 2, 1, 3, 4)
        ).reshape(B, C, S2)
        return tp.astype(dtype)

    r_b = pack(r, ml_dtypes.bfloat16)
    m_big = np.broadcast_to(m_r[:, None], (B, NS, H, W))
    m_b = pack(m_big, np.uint8)
    sel_p = pack(sel_rev, np.float32)                  # for host blend
    return r_b, m_b, sel_p


def _unpack_phase2(tb_stack, sel_p):
    """(B,128,7680) bf16 w*d in transposed layout -> (B,32,H,W) refined."""
    NT2 = S2 // 128
    t = tb_stack.astype(np.float32).reshape(B, 128, NT2, 128)
    wd = t.transpose(0, 3, 2, 1).reshape(B, 128, S2)   # back to packed layout
    r = wd + sel_p                                     # refined, packed
    r = r.reshape(B, WQ, NS, WPQ, H).transpose(0, 2, 1, 3, 4)
    r = r.reshape(B, NS, W, H).transpose(0, 1, 3, 2)   # (B, 32, Hrev, W)
    return r[:, :, ::-1, :]


def _get_runners():
    if "run1" not in _cache:
        nc1 = build_phase1()
        _cache["run1"] = _runner(nc1, N_CORES)
        nc2 = build_phase2()
        _cache["run2"] = _runner(nc2, N_CORES)
    return _cache["run1"], _cache["run2"]


def _max_masked_run(dynamic_masks):
    m = (dynamic_masks != 0)
    best = np.zeros((B, W), dtype=np.int32)
    cur = np.zeros((B, W), dtype=np.int32)
    for h in range(H - 1, -1, -1):
        cur = np.where(m[:, h, :], cur + 1, 0)
        best = np.maximum(best, cur)
    return int(best.max())


def kernel(input_features, dynamic_masks):
    input_features = np.asarray(input_features, dtype=np.float32)
    dynamic_masks = np.asarray(dynamic_masks)
    run1, run2 = _get_runners()

    # Phase 1: per-channel reductions on device (PE half + DVE half)
    xts, xn = _pack_phase1(input_features)
    if "ramp" not in _cache:
        _cache["ramp"] = _ramp_array()
    ramp = _cache["ramp"]
    in_maps1 = [{"xts": xts[b], "xn": xn[b], "rmp": ramp} for b in range(B)]
    outs1 = run1(in_maps1)
    st1_stack = np.stack([o["st1"] for o in outs1])
    idx = _select_channels(st1_stack[:, :, :SLOT],
                           st1_stack[:, :, SLOT:SLOT + (H - HSPL)],
                           st1_stack[:, :, SLOT + (H - HSPL):])

    # the single-scan propagation is exact iff no masked run >= 33
    assert _max_masked_run(dynamic_masks) <= 32, (
        "masked run of >= 33 rows: single-scan shortcut invalid for this input"
    )

    # Phase 2: propagation + blend on device (device returns w*d*rcp)
    r_b, m_b, sel_p = _pack_phase2_inputs(input_features, dynamic_masks, idx)
    in_maps2 = [{"rin": r_b[b], "msk": m_b[b]} for b in range(B)]
    outs2 = run2(in_maps2)
    tb_stack = np.stack([o["ref"] for o in outs2])
    refined = _unpack_phase2(tb_stack, sel_p)

    out = input_features.copy()
    out[:, idx] = refined
    return out


# revision 47
# speedup vs baseline: 1.0987x; 1.0064x over previous
"""Trainium2 Bass kernel for nn_GroundPropagation (optimized v3).

Phase 1 (device): host pre-swizzles 80 h-rows of x to (hw-on-partition,
channel-on-free) bf16; ACT computes sigmoid; PE runs 200 accumulating
matmuls where moving = [4 ramp cols | 128 s cols] against the s-block
stationary, yielding per-channel disparity dots, sum(s) and the Gram
diagonal ||s||^2 in one PSUM accumulator. The remaining 16 h-rows stay
channel-major: ACT sigmoid, DVE row-sums + square-accumulate. Host
combines per-core partials in f64 and ranks channels.

Phase 2 (device): layout (wq*32+ch partition, (w', h_rev) free). Host
sends r = m*(sel_below - sel) (bf16) and mask (u8); the bottom-up scan
v = m*v + r then directly yields d = prop - sel -- no subtract pass and
no sel input. DVE/Pool tree-reduce |d| for the per-channel clip max;
after a tiny PE-assisted barrier the reciprocal is folded into a
diagonal-matrix PE transpose (y = d^T * diag(rcp), bf16 PSUM), ACT
copies y to SBUF, DVE/Pool tree-max over channels gives w, ACT
broadcasts it, DVE blends w*y. Host multiplies by den_c = 0.3*max|d_c|,
adds sel and scatters.
"""

import sys

sys.path.insert(0, "/opt/trn_rl_repo")

import numpy as np
import ml_dtypes

B, C, H, W = 8, 128, 96, 320
HW = H * W                  # 30720
NSEL = 16
NS = 2 * NSEL               # 32 selected channels
CLIP = 0.3
EPS = 1e-6
N_CORES = 8

# phase 1 split: first HSPL h-rows transposed for PE, rest channel-major
HSPL = 72                   # h rows in the transposed (PE) half
HWT = HSPL * W              # 23040 elems in PE half
JBS = [36, 36, 36, 36, 24, 12]   # j-blocks per transposed chunk (180 total)
NCHT = len(JBS)
SLOT = 132                  # moving width: 4 ramp + 128 s
HRS = [6, 6, 6, 6]          # h rows per channel-major chunk (24 total)
WARMUP = 40                 # PE p-state warmup matmuls
ORDER = ["j0", "n0", "j1", "n1", "j2", "n2", "j3", "n3", "j4", "j5"]
NCHN = len(HRS)
# phase 2
WQ = 4                      # w-quarters; partition p = wq*32 + ch
WPQ = W // WQ               # 80 columns per quarter
S2 = WPQ * H                # 7680 free elems per partition
CHS = [960, 1920, 1920, 1920, 960]    # stage-I chunk lengths (scan grain)
NCH2 = len(CHS)
NSUP = 5                    # stage-II super-chunks
SUP = S2 // NSUP            # 1536

_cache = {}


def _runner(nc, n_cores):
    """Build a cached jitted callable for this Bass program via PJRT."""
    import jax
    from concourse import mybir
    from concourse.bass2jax import (
        _bass_exec_p,
        install_neuronx_cc_hook,
        partition_id_tensor,
    )
    from jax.sharding import Mesh, PartitionSpec
    from jax.experimental.shard_map import shard_map

    install_neuronx_cc_hook()
    partition_name = nc.partition_id_tensor.name if nc.partition_id_tensor else None

    in_names, out_names, out_avals = [], [], []
    for alloc in nc.m.functions[0].allocations:
        if not isinstance(alloc, mybir.MemoryLocationSet):
            continue
        name = alloc.memorylocations[0].name
        if alloc.kind == "ExternalInput":
            if name != partition_name:
                in_names.append(name)
        elif alloc.kind == "ExternalOutput":
            out_names.append(name)
            out_avals.append(
                jax.core.ShapedArray(
                    tuple(alloc.tensor_shape), mybir.dt.np(alloc.dtype)
                )
            )
    n_params = len(in_names)
    n_outs = len(out_avals)
    all_names = in_names + out_names + ([partition_name] if partition_name else [])
    donate = tuple(range(n_params, n_params + n_outs))

    def _body(*args):
        operands = list(args)
        if partition_name is not None:
            operands.append(partition_id_tensor())
        outs = _bass_exec_p.bind(
            *operands,
            out_avals=tuple(out_avals),
            in_names=tuple(all_names),
            out_names=tuple(out_names),
            lowering_input_output_aliases=(),
            sim_require_finite=True,
            sim_require_nnan=True,
            nc=nc,
        )
        return tuple(outs)

    devices = jax.devices()[:n_cores]
    mesh = Mesh(np.asarray(devices), ("core",))
    in_specs = (PartitionSpec("core"),) * (n_params + n_outs)
    out_specs = (PartitionSpec("core"),) * n_outs
    sharded = jax.jit(
        shard_map(
            _body, mesh=mesh, in_specs=in_specs, out_specs=out_specs, check_rep=False
        ),
        donate_argnums=donate,
        keep_unused=True,
    )

    def run(in_maps):
        concat_in = [
            np.concatenate([np.asarray(m[name]) for m in in_maps], axis=0)
            for name in in_names
        ]
        zeros = [
            np.zeros((n_cores * a.shape[0], *a.shape[1:]), a.dtype) for a in out_avals
        ]
        out_arrs = sharded(*concat_in, *zeros)
        return [
            {
                name: np.asarray(out_arrs[i]).reshape(
                    n_cores, *out_avals[i].shape
                )[c]
                for i, name in enumerate(out_names)
            }
            for c in range(n_cores)
        ]

    return run


def build_phase1():
    from contextlib import ExitStack

    import concourse.tile as tile
    from concourse import bacc, mybir

    f32 = mybir.dt.float32
    bf16 = mybir.dt.bfloat16
    Alu = mybir.AluOpType
    Act = mybir.ActivationFunctionType
    nc = bacc.Bacc("TRN2", target_bir_lowering=False, debug=False,
                   num_devices=N_CORES)
    xts = nc.dram_tensor("xts", (C, HWT), bf16, kind="ExternalInput").ap()
    xn = nc.dram_tensor("xn", (C, HW - HWT), bf16, kind="ExternalInput").ap()
    rmp = nc.dram_tensor("rmp", (C, HWT // 128 * 4), bf16,
                         kind="ExternalInput").ap()
    # single packed output: [gram SLOT | rowsums | per-chunk ssq]
    OUTW = SLOT + (H - HSPL) + NCHN
    st1 = nc.dram_tensor("st1", (C, OUTW), f32, kind="ExternalOutput").ap()

    with tile.TileContext(nc) as tc, ExitStack() as ctx:
        px = ctx.enter_context(tc.tile_pool(name="px", bufs=2))
        pst = ctx.enter_context(tc.tile_pool(name="pst", bufs=2))
        pxn = ctx.enter_context(tc.tile_pool(name="pxn", bufs=2))
        psn = ctx.enter_context(tc.tile_pool(name="psn", bufs=2))
        psq = ctx.enter_context(tc.tile_pool(name="psq", bufs=2))
        psm = ctx.enter_context(tc.tile_pool(name="psm", bufs=1))
        pps = ctx.enter_context(tc.tile_pool(name="pps", bufs=1, space="PSUM"))

        rt = psm.tile([C, HWT // 128 * 4], bf16)
        nc.sync.dma_start(rt[:], rmp[:])
        rtv = rt[:].rearrange("p (j r) -> p j r", r=4)
        outb = psm.tile([C, SLOT + (H - HSPL) + NCHN], f32)
        osb = outb[:, 0:SLOT]
        rowsb = outb[:, SLOT:SLOT + (H - HSPL)]
        ssqsb = outb[:, SLOT + (H - HSPL):]
        acc = pps.tile([C, SLOT], f32)
        jofs = [sum(JBS[:k]) for k in range(NCHT + 1)]
        hofs = [sum(HRS[:k]) for k in range(NCHN + 1)]
        NTOT = sum(JBS)
        # interleave: DVE work (n chunks) starts early; remaining j chunks
        # taper so the PE tail drains alongside the last ACT ops
        order = list(ORDER)
        for key in order:
            i = int(key[1:])
            if key[0] == "n":
                hr = HRS[i]
                cw = hr * W
                xcn = pxn.tile([C, cw], bf16, tag="xcn",
                               padded_shape=[C, max(HRS) * W])
                nc.sync.dma_start(xcn[:], xn[:, hofs[i] * W:hofs[i + 1] * W])
                sn_ = psn.tile([C, cw], bf16, tag="sn",
                               padded_shape=[C, max(HRS) * W])
                nc.scalar.activation(sn_[:], xcn[:], Act.Sigmoid)
                nc.vector.tensor_reduce(
                    rowsb[:, hofs[i]:hofs[i + 1]],
                    sn_[:].rearrange("p (h w) -> p h w", w=W),
                    mybir.AxisListType.X, Alu.add)
                sq = psq.tile([C, cw], bf16, tag="sq",
                              padded_shape=[C, max(HRS) * W])
                nc.vector.scalar_tensor_tensor(
                    sq[:], sn_[:], 1.0, sn_[:], op0=Alu.mult, op1=Alu.mult,
                    accum_out=ssqsb[:, i:i + 1])
            else:
                jb = JBS[i]
                xc = px.tile([C, jb * 128], bf16, tag="xc",
                             padded_shape=[C, max(JBS) * 128])
                nc.sync.dma_start(xc[:], xts[:, jofs[i] * 128:jofs[i + 1] * 128])
                # split only the tail chunks: separate tiles per half so PE
                # starts on the first half while ACT does the second
                halves = ((0, jb // 2, "sta"), (jb // 2, jb, "stb")) \
                    if i >= NCHT - 2 else ((0, jb, "sta"),)
                for (a, b, tg) in halves:
                    hb = b - a
                    st = pst.tile([C, hb * SLOT], bf16, tag=tg,
                                  padded_shape=[C, max(JBS) * SLOT])
                    stv = st[:].rearrange("p (j s) -> p j s", s=SLOT)
                    nc.vector.tensor_copy(stv[:, :, 0:4],
                                          rtv[:, jofs[i] + a:jofs[i] + b, :])
                    nc.scalar.activation(
                        stv[:, :, 4:SLOT],
                        xc[:].rearrange("p (j c) -> p j c", c=128)[:, a:b],
                        Act.Sigmoid,
                    )
                    for j in range(hb):
                        g = jofs[i] + a + j
                        nc.tensor.matmul(
                            acc[:],
                            st[:, j * SLOT + 4:(j + 1) * SLOT],
                            st[:, j * SLOT:(j + 1) * SLOT],
                            start=(g == 0), stop=(g == NTOT - 1),
                        )
        nc.vector.tensor_copy(osb, acc[:])
        nc.sync.dma_start(st1[:], outb[:])
    nc.compile()
    return nc


def build_phase2():
    from contextlib import ExitStack

    import concourse.tile as tile
    from concourse import bacc, mybir
    from concourse.masks import make_identity

    f32 = mybir.dt.float32
    bf16 = mybir.dt.bfloat16
    u8 = mybir.dt.uint8
    Alu = mybir.AluOpType
    Act = mybir.ActivationFunctionType
    nc = bacc.Bacc("TRN2", target_bir_lowering=False, debug=False,
                   num_devices=N_CORES)
    rin = nc.dram_tensor("rin", (C, S2), bf16, kind="ExternalInput").ap()
    msk = nc.dram_tensor("msk", (C, S2), u8, kind="ExternalInput").ap()
    ref = nc.dram_tensor("ref", (C, S2), bf16, kind="ExternalOutput").ap()
    NT = SUP // 128             # transpose tiles per super-chunk

    with tile.TileContext(nc) as tc, ExitStack() as ctx:
        pools = {}
        for name, bufs in [("r", 5), ("u", 1), ("m", 5), ("ad", 2), ("t1", 2),
                           ("t2", 2), ("ysb", 2), ("c1", 2), ("c2", 2),
                           ("wr", 2), ("wex", 2), ("tb", 2), ("sm", 1)]:
            pools[name] = ctx.enter_context(tc.tile_pool(name=name, bufs=bufs))
        ppsy = ctx.enter_context(tc.tile_pool(name="ppsy", bufs=2, space="PSUM"))
        ppst = ctx.enter_context(tc.tile_pool(name="ppst", bufs=1, space="PSUM"))
        psm = pools["sm"]

        identf = psm.tile([C, C], f32)
        make_identity(nc, identf[:])
        identb = psm.tile([C, C], bf16)
        make_identity(nc, identb[:])
        mxp = psm.tile([C, NCH2], f32)
        mxr = psm.tile([C, 1], f32)
        Mc = psm.tile([1, NS], f32)
        den = psm.tile([1, NS], f32)
        rc = psm.tile([1, NS], f32)
        rc4 = psm.tile([1, C], bf16)
        one1 = psm.tile([1, 1], bf16)
        nc.vector.memset(one1[:], 1.0)
        rcp = psm.tile([C, 1], f32)
        diag = psm.tile([C, C], bf16)
        ut = pools["u"].tile([C, S2], bf16)

        # ---- stage I: per chunk scan -> d, and |d| absmax tree ----
        # leading chunks are small so the first scan starts early; the raw
        # uT transposes for the first two supers run here (no rcp needed),
        # warming the PE and shrinking the post-barrier fill
        utps = {}
        done = 0
        next_ut = 0
        for k, ch in enumerate(CHS):
            ksl = slice(done, done + ch)
            mt = pools["m"].tile([C, ch], u8, tag="m",
                                 padded_shape=[C, max(CHS)])
            nc.sync.dma_start(mt[:], msk[:, ksl])
            rt_ = pools["r"].tile([C, ch], bf16, tag="r",
                                  padded_shape=[C, max(CHS)])
            nc.sync.dma_start(rt_[:], rin[:, ksl])
            # d = scan: state = m*state + r  (bottom-up, per column)
            nc.vector.tensor_tensor_scan(
                ut[:, ksl], mt[:], rt_[:], 0.0,
                op0=Alu.mult, op1=Alu.add)
            done += ch
            ad = pools["ad"].tile([C, ch], bf16, tag="ad",
                                  padded_shape=[C, max(CHS)])
            nc.scalar.activation(ad[:], ut[:, ksl], Act.Abs)
            t1 = pools["t1"].tile([C, ch // 2], bf16, tag="t1",
                                  padded_shape=[C, max(CHS) // 2])
            nc.vector.tensor_tensor(
                t1[:], ad[:, :ch // 2], ad[:, ch // 2:], Alu.max)
            t2 = pools["t2"].tile([C, ch // 4], bf16, tag="t2",
                                  padded_shape=[C, max(CHS) // 4])
            nc.vector.tensor_tensor(
                t2[:], t1[:, :ch // 4], t1[:, ch // 4:], Alu.max)
            nc.vector.tensor_reduce(
                mxp[:, k:k + 1], t2[:], mybir.AxisListType.X, Alu.max)


        # ---- barrier (PE-assisted, no DMA) ----
        nc.vector.tensor_reduce(mxr[:], mxp[:], mybir.AxisListType.X, Alu.max)
        # barrier PE outputs live in slices of the (bufs=1) y psum ring
        ybar = ppsy.tile([C, SUP], f32, tag="y")
        mrow_ps = ybar[0:1, 0:C]
        nc.tensor.transpose(mrow_ps, mxr[:], identf[:])
        nc.vector.tensor_reduce(
            Mc[:], mrow_ps.rearrange("o (q c) -> o c q", q=WQ),
            mybir.AxisListType.X, Alu.max)
        # m==0 -> |d|==0 everywhere for that channel, so any finite rcp works
        nc.vector.tensor_scalar(den[:], Mc[:], CLIP, 1e-20,
                                op0=Alu.mult, op1=Alu.max)
        nc.vector.reciprocal(rc[:], den[:])
        nc.vector.tensor_copy(
            rc4[:].rearrange("o (q c) -> o q c", q=WQ),
            rc[:].unsqueeze(1).broadcast_to((1, WQ, NS)))
        rcp_ps = ybar[:, 128:129]
        nc.tensor.matmul(rcp_ps, rc4[:], one1[:], start=True, stop=True)
        nc.vector.tensor_copy(rcp[:], rcp_ps)
        # diag(rcp) for the scaled PE transpose
        nc.vector.tensor_scalar(diag[:], identf[:], rcp[:], None, op0=Alu.mult)

        # ---- stage II per super-chunk: scaled transpose, chanmax, blend ----
        def emit_y(g):
            yps = ppsy.tile([C, SUP], f32, tag="y")
            for t in range(NT):
                o = g * SUP + t * 128
                # y = u^T @ diag(rcp): transpose + per-channel scale in one
                nc.tensor.matmul(yps[:, t * 128:(t + 1) * 128],
                                 ut[:, o:o + 128], diag[:],
                                 start=True, stop=True)
            ysb = pools["ysb"].tile([C, SUP], bf16, tag="ysb")
            nc.scalar.activation(ysb[:], yps[:], Act.Abs)
            return ysb

        NG = SUP // NS
        ysbs = {0: emit_y(0)}
        for g in range(NSUP):
            gsl = slice(g * SUP, (g + 1) * SUP)
            if g + 1 < NSUP:
                ysbs[g + 1] = emit_y(g + 1)
            ysb = ysbs.pop(g)
            yv = ysb[:].rearrange("p (g c) -> p g c", c=NS)
            c1 = pools["c1"].tile([C, NG * 16], bf16, tag="c1")
            nc.vector.tensor_tensor(
                c1[:].rearrange("p (g c) -> p g c", c=16),
                yv[:, :, 0:16], yv[:, :, 16:32], Alu.max)
            c1v = c1[:].rearrange("p (g c) -> p g c", c=16)
            c2 = pools["c2"].tile([C, NG * 8], bf16, tag="c2")
            nc.vector.tensor_tensor(
                c2[:].rearrange("p (g c) -> p g c", c=8),
                c1v[:, :, 0:8], c1v[:, :, 8:16], Alu.max)
            wr = pools["wr"].tile([C, NG], bf16, tag="wr")
            nc.vector.tensor_reduce(
                wr[:], c2[:].rearrange("p (g c) -> p g c", c=8),
                mybir.AxisListType.X, Alu.max)
            nc.vector.tensor_scalar(wr[:], wr[:], 1.0, None, op0=Alu.min)
            # raw d^T just-in-time for the blend
            utp = ppst.tile([C, SUP], bf16, tag="uT")
            for t in range(NT):
                o = g * SUP + t * 128
                nc.tensor.transpose(utp[:, t * 128:(t + 1) * 128],
                                    ut[:, o:o + 128], identb[:])
            # broadcast w over the 32 channel slots and blend with raw d^T;
            # the last super streams out in halves to shrink the tail
            wex = pools["wex"].tile([C, SUP], bf16, tag="wex")
            tb = pools["tb"].tile([C, SUP], bf16, tag="tb")
            parts = 2 if g == NSUP - 1 else 1
            hw_ = SUP // parts
            hg = NG // parts
            for h in range(parts):
                hsl = slice(h * hw_, (h + 1) * hw_)
                nc.scalar.activation(
                    wex[:, hsl].rearrange("p (g c) -> p g c", c=NS),
                    wr[:, h * hg:(h + 1) * hg].unsqueeze(-1)
                    .broadcast_to((C, hg, NS)),
                    Act.Copy)
                nc.vector.tensor_tensor(tb[:, hsl], wex[:, hsl], utp[:, hsl],
                                        Alu.mult)
                nc.sync.dma_start(
                    ref[:, g * SUP + h * hw_:g * SUP + (h + 1) * hw_],
                    tb[:, hsl])
    nc.compile()
    return nc


# ---------------- host side ----------------

def _dispfull_f32():
    disp = np.linspace(0.1, 1.0, H).astype(np.float32)
    return np.repeat(disp, W)                       # (HW,) hw = h*W + w


def _ramp_array():
    """(128, HWT//128 * 4) bf16: per j-block cols [disp_hi, disp_lo, 1, 0]."""
    df = _dispfull_f32()[:HWT]
    hi = df.astype(ml_dtypes.bfloat16)
    lo = (df - hi.astype(np.float32)).astype(ml_dtypes.bfloat16)
    nj = HWT // 128
    r = np.zeros((128, nj, 4), dtype=ml_dtypes.bfloat16)
    r[:, :, 0] = hi.reshape(nj, 128).T
    r[:, :, 1] = lo.reshape(nj, 128).T
    r[:, :, 2] = np.asarray(1.0, ml_dtypes.bfloat16)
    return np.ascontiguousarray(r).reshape(128, nj * 4)


def _pack_phase1(x):
    """(B,C,H,W) f32 -> transposed half (B,128,HWT) + normal half, bf16."""
    xb = x.astype(ml_dtypes.bfloat16)
    xt_half = xb[:, :, :HSPL, :].reshape(B, C, HWT // 128, 128)
    xts = np.ascontiguousarray(xt_half.transpose(0, 3, 2, 1)).reshape(B, 128, HWT)
    xn = np.ascontiguousarray(xb[:, :, HSPL:, :]).reshape(B, C, HW - HWT)
    return xts, xn


def _select_channels(st1_stack, rows_stack, ssqn_stack):
    """Combine per-core phase-1 outputs in f64 and rank channels."""
    blk = st1_stack.astype(np.float64)                     # (B, C, SLOT)
    dot_disp = blk[:, :, 0].sum(axis=0) + blk[:, :, 1].sum(axis=0)
    sum_s = blk[:, :, 2].sum(axis=0)
    g = blk[:, :, 4:]                                      # (B, C, C)
    ssq = np.einsum('bcc->c', g, optimize=True)
    # channel-major-half contributions
    rows = rows_stack.astype(np.float64).sum(axis=0)       # (C, H-HSPL)
    disp = np.linspace(0.1, 1.0, H).astype(np.float32).astype(np.float64)
    dot_disp += rows @ disp[HSPL:]
    sum_s += rows.sum(axis=1)
    ssq += ssqn_stack.astype(np.float64).sum(axis=(0, 2))
    dot_depth = sum_s - dot_disp
    df = _dispfull_f32().astype(np.float64)
    vn_disp = np.sqrt(B * (df @ df))
    vn_depth = np.sqrt(B * ((1.0 - df) @ (1.0 - df)))
    sn = np.maximum(np.sqrt(ssq), EPS)
    cos_disp = dot_disp / (sn * vn_disp)
    cos_depth = dot_depth / (sn * vn_depth)
    disp_idx = np.argsort(-cos_disp, kind="stable")[:NSEL]
    depth_idx = np.argsort(-cos_depth, kind="stable")[:NSEL]
    return np.concatenate([disp_idx, depth_idx])


def _pack_phase2_inputs(input_features, dynamic_masks, idx):
    """Pack r = m*(sel_below - sel) and mask into per-core (128, 7680)
    bf16/u8 layout: partition p = wq*32 + ch, free t = w'*96 + (95 - h)."""
    sel = input_features[:, idx]                       # (B, 32, H, W) f32
    m_r = (dynamic_masks[:, ::-1, :] != 0)             # (B, Hrev, W) bool
    m_r = m_r.copy()
    m_r[:, 0, :] = False            # force reset at each column's bottom row

    sel_rev = sel[:, :, ::-1, :]                       # (B, 32, Hrev, W)
    dsel = np.zeros_like(sel_rev)
    dsel[:, :, 1:] = sel_rev[:, :, :-1] - sel_rev[:, :, 1:]
    r = np.where(m_r[:, None], dsel, 0.0)              # (B, 32, Hrev, W)

    def pack(t, dtype):                                # (B,32,Hrev,W) -> (B,128,S2)
        tt = t.transpose(0, 1, 3, 2)                   # (B, 32, W, Hrev)
        tp = np.ascontiguousarray(
            tt.reshape(B, NS, WQ, WPQ, H).transpose(0, 2, 1, 3, 4)
        ).reshape(B, C, S2)
        return tp.astype(dtype)

    r_b = pack(r, ml_dtypes.bfloat16)
    m_big = np.broadcast_to(m_r[:, None], (B, NS, H, W))
    m_b = pack(m_big, np.uint8)
    sel_p = pack(sel_rev, np.float32)                  # for host blend
    return r_b, m_b, sel_p


def _unpack_phase2(tb_stack, sel_p):
    """(B,128,7680) bf16 w*d in transposed layout -> (B,32,H,W) refined."""
    NT2 = S2 // 128
    t = tb_stack.astype(np.float32).reshape(B, 128, NT2, 128)
    wd = t.transpose(0, 3, 2, 1).reshape(B, 128, S2)   # back to packed layout
    r = wd + sel_p                                     # refined, packed
    r = r.reshape(B, WQ, NS, WPQ, H).transpose(0, 2, 1, 3, 4)
    r = r.reshape(B, NS, W, H).transpose(0, 1, 3, 2)   # (B, 32, Hrev, W)
    return r[:, :, ::-1, :]


def _get_runners():
    if "run1" not in _cache:
        nc1 = build_phase1()
        _cache["run1"] = _runner(nc1, N_CORES)
        nc2 = build_phase2()
        _cache["run2"] = _runner(nc2, N_CORES)
    return _cache["run1"], _cache["run2"]


def _max_masked_run(dynamic_masks):
    m = (dynamic_masks != 0)
    best = np.zeros((B, W), dtype=np.int32)
    cur = np.zeros((B, W), dtype=np.int32)
    for h in range(H - 1, -1, -1):
        cur = np.where(m[:, h, :], cur + 1, 0)
        best = np.maximum(best, cur)
    return int(best.max())


def kernel(input_features, dynamic_masks):
    input_features = np.asarray(input_features, dtype=np.float32)
    dynamic_masks = np.asarray(dynamic_masks)
    run1, run2 = _get_runners()

    # Phase 1: per-channel reductions on device (PE half + DVE half)
    xts, xn = _pack_phase1(input_features)
    if "ramp" not in _cache:
        _cache["ramp"] = _ramp_array()
    ramp = _cache["ramp"]
    in_maps1 = [{"xts": xts[b], "xn": xn[b], "rmp": ramp} for b in range(B)]
    outs1 = run1(in_maps1)
    st1_stack = np.stack([o["st1"] for o in outs1])
    idx = _select_channels(st1_stack[:, :, :SLOT],
                           st1_stack[:, :, SLOT:SLOT + (H - HSPL)],
                           st1_stack[:, :, SLOT + (H - HSPL):])

    # the single-scan propagation is exact iff no masked run >= 33
    assert _max_masked_run(dynamic_masks) <= 32, (
        "masked run of >= 33 rows: single-scan shortcut invalid for this input"
    )

    # Phase 2: propagation + blend on device (device returns w*d*rcp)
    r_b, m_b, sel_p = _pack_phase2_inputs(input_features, dynamic_masks, idx)
    in_maps2 = [{"rin": r_b[b], "msk": m_b[b]} for b in range(B)]
    outs2 = run2(in_maps2)
    tb_stack = np.stack([o["ref"] for o in outs2])
    refined = _unpack_phase2(tb_stack, sel_p)

    out = input_features.copy()
    out[:, idx] = refined
    return out
